# revision 4
# baseline (speedup 1.0000x reference)
"""BondMessagePassing kernel for 8 Trainium2 NeuronCores.

Edge-sharded data parallelism: 512 edges per core. Per layer:
  - node segment-sum via matmul with host-built one-hot + AllReduce
  - gather + residual term via one fused matmul (B = [A^T; -diag(deg)])
  - full-sequence MHA over 4096 edges: each core computes its 512 query
    rows against the AllGathered K/V of all cores
Linears run in transposed-activation layout so weights are natural lhsT;
PE transposes switch layouts where LayerNorm/segment ops need row layout.
"""

import numpy as np
import ml_dtypes

import concourse.bass as bass
import concourse.tile as tile
import concourse.mybir as mybir
from concourse import bacc
from concourse.bass_utils import run_bass_kernel_spmd
from concourse.masks import make_identity

F32 = mybir.dt.float32
BF16 = mybir.dt.bfloat16
AF = mybir.ActivationFunctionType
ALU = mybir.AluOpType
BFNP = ml_dtypes.bfloat16

NC = 8          # cores
P = 128         # partitions
NN = 1024       # nodes
E = 4096        # edges
EL = E // NC    # edges per core (512)
H = 256         # hidden
BD = 64         # bond dim
NH = 8          # heads
D = H // NH     # head dim (32)
L = 3           # layers
HK = H // P     # 2  K-chunks per 256
EC = EL // P    # 4  edge chunks per core
NT = NN // P    # 8  node tiles
KT = E // P     # 32 k-tiles (global edges)
M6 = 3 * H // P  # 6 qkv out tiles
JB = NT + EC    # 12 K-chunks of the fused r matmul
AGW = 1024 + EC * NH * 33  # 2080 allgather row width (K^T 1024 + V_aug 1056)


def _build():
    nc = bacc.Bacc(None, target_bir_lowering=False, num_devices=NC)

    di = {}
    def din(name, shape, dtype):
        di[name] = nc.dram_tensor(name, shape, dtype, kind="ExternalInput")
        return di[name]

    din("bondT", [BD, EL], BF16)
    din("Amat", [P, EC, NN], BF16)
    din("Bmat", [P, JB, EL], BF16)
    din("wemb", [BD, H], BF16)
    din("bemb", [P, HK], F32)
    din("wh", [P, HK, H], BF16)
    din("bh", [P, HK], F32)
    din("inw", [P, L, HK, 3 * H], BF16)
    din("inb", [P, L, M6], F32)
    din("outw", [P, L, HK, H], BF16)
    din("upw", [P, L, HK, H], BF16)
    din("upb2", [L, H], F32)
    din("ln1g", [L, H], F32)
    din("ln1b", [L, H], F32)
    din("ln2g", [L, H], F32)
    din("ln2b", [L, H], F32)
    hout = nc.dram_tensor("hout", [EL, H], F32, kind="ExternalOutput")

    rg = [list(range(NC))]

    with tile.TileContext(nc) as tc:
        with (
            tc.tile_pool(name="const", bufs=1) as const,
            tc.tile_pool(name="sb", bufs=2) as sb,
            tc.tile_pool(name="kv", bufs=1) as kv,
            tc.tile_pool(name="ptp", bufs=4) as ptp,
            tc.tile_pool(name="pmm", bufs=3, space="PSUM") as pmm,
            tc.tile_pool(name="pacc", bufs=2, space="PSUM") as pacc,
            tc.tile_pool(name="ptr", bufs=2, space="PSUM") as ptr,
            tc.tile_pool(name="pbc", bufs=1, space="PSUM") as pbc,
            tc.tile_pool(name="dram", bufs=1, space="DRAM") as dram,
        ):
            # ---- load constants ----
            bondT_sb = const.tile([BD, EL], BF16)
            nc.sync.dma_start(bondT_sb[:], di["bondT"][:])
            A_sb = const.tile([P, EC, NN], BF16)
            nc.sync.dma_start(A_sb[:], di["Amat"][:])
            B_sb = const.tile([P, JB, EL], BF16)
            nc.sync.dma_start(B_sb[:], di["Bmat"][:])
            wemb_sb = const.tile([BD, H], BF16)
            nc.sync.dma_start(wemb_sb[:], di["wemb"][:])
            bemb_sb = const.tile([P, HK], F32)
            nc.sync.dma_start(bemb_sb[:], di["bemb"][:])
            wh_sb = const.tile([P, HK, H], BF16)
            nc.sync.dma_start(wh_sb[:], di["wh"][:])
            bh_sb = const.tile([P, HK], F32)
            nc.sync.dma_start(bh_sb[:], di["bh"][:])
            inw_sb = const.tile([P, L, HK, 3 * H], BF16)
            nc.sync.dma_start(inw_sb[:], di["inw"][:])
            inb_sb = const.tile([P, L, M6], F32)
            nc.sync.dma_start(inb_sb[:], di["inb"][:])
            outw_sb = const.tile([P, L, HK, H], BF16)
            nc.sync.dma_start(outw_sb[:], di["outw"][:])
            upw_sb = const.tile([P, L, HK, H], BF16)
            nc.sync.dma_start(upw_sb[:], di["upw"][:])

            def bcast_load(name):
                t = const.tile([P, L, H], F32, name=f"{name}_bc")
                src = di[name][:]
                bap = bass.AP(
                    tensor=src.tensor,
                    offset=src.offset,
                    ap=[[0, P]] + [list(x) for x in src.ap],
                )
                nc.sync.dma_start(t[:], bap)
                return t

            upb2_bc = bcast_load("upb2")
            ln1g_bc = bcast_load("ln1g")
            ln1b_bc = bcast_load("ln1b")
            ln2g_bc = bcast_load("ln2g")
            ln2b_bc = bcast_load("ln2b")

            ident_bf = const.tile([P, P], BF16)
            make_identity(nc, ident_bf[:])
            ones_f = const.tile([1, D], F32)
            nc.vector.memset(ones_f[:], 1.0)
            eps_sb = const.tile([P, 1], F32)
            nc.vector.memset(eps_sb[:], 1e-5)

            def transpose_128(dst_ap, src_ap):
                pst = ptr.tile([P, P], BF16, tag="tr")
                nc.tensor.transpose(pst[:], src_ap, ident_bf[:])
                nc.vector.tensor_copy(dst_ap, pst[:])

            def layer_norm_apply(dst_ap, x_f32, g_ap, b_ap, tmp_tag):
                """dst = (x - mean(x)) * rsqrt(var+eps) * g + b  (per partition row)"""
                stats = sb.tile([P, 6], F32, tag="stats", name="stats")
                nc.vector.bn_stats(stats[:], x_f32)
                mv = sb.tile([P, 2], F32, tag="mv", name="mv")
                nc.vector.bn_aggr(mv[:], stats[:])
                nc.scalar.activation(mv[:, 1:2], mv[:, 1:2], AF.Sqrt, bias=eps_sb[:])
                nc.vector.reciprocal(mv[:, 1:2], mv[:, 1:2])
                xc = sb.tile([P, H], F32, tag=tmp_tag, name=tmp_tag)
                nc.vector.tensor_scalar(
                    xc[:], x_f32, mv[:, 0:1], mv[:, 1:2],
                    op0=ALU.subtract, op1=ALU.mult,
                )
                nc.vector.tensor_mul(xc[:], xc[:], g_ap)
                nc.vector.tensor_add(dst_ap, xc[:], b_ap)

            # ---- embedding: h = gelu(bond @ W_emb + b_emb) @ W_h + b_h ----
            g1 = sb.tile([P, HK, EL], BF16, name="g1")
            for m in range(HK):
                ps = pmm.tile([P, EL], F32, tag="mm", name="ps_e")
                nc.tensor.matmul(
                    ps[:], wemb_sb[:, m * P:(m + 1) * P], bondT_sb[:],
                    start=True, stop=True,
                )
                nc.scalar.activation(
                    g1[:, m, :], ps[:], AF.Gelu, bias=bemb_sb[:, m:m + 1]
                )
            hT = sb.tile([P, HK, EL], BF16, name="hT")
            for m in range(HK):
                ps = pmm.tile([P, EL], F32, tag="mm", name="ps_h")
                for k in range(HK):
                    nc.tensor.matmul(
                        ps[:], wh_sb[:, k, m * P:(m + 1) * P], g1[:, k, :],
                        start=(k == 0), stop=(k == HK - 1),
                    )
                nc.vector.tensor_scalar_add(hT[:, m, :], ps[:], bh_sb[:, m:m + 1])
            h_nat = sb.tile([P, EC, H], BF16, name="h_nat")
            for m in range(HK):
                for c in range(EC):
                    transpose_128(
                        h_nat[:, c, m * P:(m + 1) * P],
                        hT[:, m, c * P:(c + 1) * P],
                    )

            # ---- layers ----
            for t in range(L):
                # A. partial segment-sum over local edges, AllReduce
                ar_in = dram.tile([NN, H], F32, name=f"ar_in{t}")
                ar_out = dram.tile([NN, H], F32, addr_space="Shared", name=f"ar_out{t}")
                for i in range(NT):
                    ps = pmm.tile([P, EL], F32, tag="mm", name="ps_s")
                    for c in range(EC):
                        nc.tensor.matmul(
                            ps[:, :H], A_sb[:, c, i * P:(i + 1) * P], h_nat[:, c, :],
                            start=(c == 0), stop=(c == EC - 1),
                        )
                    s32 = sb.tile([P, H], F32, tag="s32", name="s32")
                    nc.vector.tensor_copy(s32[:], ps[:, :H])
                    nc.sync.dma_start(ar_in[i * P:(i + 1) * P, :], s32[:])
                nc.gpsimd.collective_compute(
                    "AllReduce", ALU.add, replica_groups=rg,
                    ins=[ar_in[:]], outs=[ar_out[:]],
                )
                s_bf = sb.tile([P, NT, H], BF16, name="s_bf")
                for i in range(NT):
                    sf = sb.tile([P, H], F32, tag="sf", name="sf")
                    nc.sync.dma_start(sf[:], ar_out[i * P:(i + 1) * P, :])
                    nc.vector.tensor_copy(s_bf[:, i, :], sf[:])

                # B. r = S[tgt] - deg[tgt]*h  (fused matmul), keep f32
                r_nat = sb.tile([P, EC, H], F32, name="r_nat")
                xn_bf = sb.tile([P, EC, H], BF16, name="xn_bf")
                for m in range(EC):
                    ps = pmm.tile([P, EL], F32, tag="mm", name="ps_r")
                    for j in range(JB):
                        rhs = s_bf[:, j, :] if j < NT else h_nat[:, j - NT, :]
                        nc.tensor.matmul(
                            ps[:, :H], B_sb[:, j, m * P:(m + 1) * P], rhs,
                            start=(j == 0), stop=(j == JB - 1),
                        )
                    nc.vector.tensor_copy(r_nat[:, m, :], ps[:, :H])
                    # C. LN1 -> xn (bf16)
                    layer_norm_apply(
                        xn_bf[:, m, :], r_nat[:, m, :],
                        ln1g_bc[:, t, :], ln1b_bc[:, t, :], "xn32",
                    )

                # D. xn^T
                xnT = sb.tile([P, HK, EL], BF16, name="xnT")
                for c in range(EC):
                    for hf in range(HK):
                        transpose_128(
                            xnT[:, hf, c * P:(c + 1) * P],
                            xn_bf[:, c, hf * P:(hf + 1) * P],
                        )

                # E. in-proj: qkv^T = in_w^T @ xn^T + in_b
                QT = sb.tile([P, HK, EL], BF16, name="QT")
                KTl = sb.tile([P, HK, EL], BF16, name="KTl")
                VTl = sb.tile([P, HK, EL], BF16, name="VTl")
                dests = [(QT, 0), (QT, 1), (KTl, 0), (KTl, 1), (VTl, 0), (VTl, 1)]
                for m in range(M6):
                    ps = pmm.tile([P, EL], F32, tag="mm", name="ps_q")
                    for k in range(HK):
                        nc.tensor.matmul(
                            ps[:], inw_sb[:, t, k, m * P:(m + 1) * P], xnT[:, k, :],
                            start=(k == 0), stop=(k == HK - 1),
                        )
                    dt_, idx = dests[m]
                    nc.vector.tensor_scalar_add(
                        dt_[:, idx, :], ps[:], inb_sb[:, t, m:m + 1]
                    )

                # F. v natural + ones column (V_aug)
                vnat = sb.tile([P, EC, NH, 33], BF16, name="vnat")
                for hf in range(HK):
                    for c in range(EC):
                        pst = ptr.tile([P, P], BF16, tag="tr", name="pst_v")
                        nc.tensor.transpose(
                            pst[:], VTl[:, hf, c * P:(c + 1) * P], ident_bf[:]
                        )
                        nc.vector.tensor_copy(
                            vnat[:, c, hf * 4:(hf + 1) * 4, 0:32],
                            pst[:].rearrange("p (a b) -> p a b", a=4),
                        )
                nc.vector.memset(vnat[:, :, :, 32:33], 1.0)

                # G. AllGather K^T and V_aug
                ag_in = dram.tile([P, AGW], BF16, name=f"ag_in{t}")
                ag_out = dram.tile(
                    [P * NC, AGW], BF16, addr_space="Shared", name=f"ag_out{t}"
                )
                nc.sync.dma_start(
                    ag_in[:, 0:1024].rearrange("p (a b) -> p a b", a=HK), KTl[:]
                )
                nc.sync.dma_start(
                    ag_in[:, 1024:AGW].rearrange(
                        "p (a b c) -> p a b c", a=EC, b=NH
                    ),
                    vnat[:],
                )
                nc.gpsimd.collective_compute(
                    "AllGather", ALU.bypass, replica_groups=rg,
                    ins=[ag_in[:]], outs=[ag_out[:]],
                )
                KT_all = kv.tile([P, NC, HK, EL], BF16, name="KT_all")
                V_all = kv.tile([P, NC, EC, NH, 33], BF16, name="V_all")
                for s in range(NC):
                    nc.sync.dma_start(
                        KT_all[:, s, :, :],
                        ag_out[s * P:(s + 1) * P, 0:1024].rearrange(
                            "p (a b) -> p a b", a=HK
                        ),
                    )
                    nc.sync.dma_start(
                        V_all[:, s, :, :, :],
                        ag_out[s * P:(s + 1) * P, 1024:AGW].rearrange(
                            "p (a b c) -> p a b c", a=EC, b=NH
                        ),
                    )

                # H. attention (per head): scores^T tiles, exp, PV with ones row
                oT = sb.tile([P, HK, EL], BF16, name="oT")
                for h in range(NH):
                    hp = (h % 4) * D
                    hf = h // 4
                    ps_o = pacc.tile([33, EL], F32, tag="acc", name="ps_o")
                    for kt in range(KT):
                        s, c = divmod(kt, EC)
                        ps_s = pmm.tile([P, EL], F32, tag="mm", name="ps_sc")
                        nc.tensor.matmul(
                            ps_s[:],
                            KT_all[hp:hp + D, s, hf, c * P:(c + 1) * P],
                            QT[hp:hp + D, hf, :],
                            start=True, stop=True,
                            tile_position=(hp, 0),
                        )
                        pt = ptp.tile([P, EL], BF16, tag="pt", name="pt")
                        nc.scalar.activation(pt[:], ps_s[:], AF.Exp)
                        nc.tensor.matmul(
                            ps_o[:], V_all[:, s, c, h, 0:33], pt[:],
                            start=(kt == 0), stop=(kt == KT - 1),
                        )
                    rec = sb.tile([1, EL], F32, tag="rec", name="rec")
                    nc.vector.reciprocal(rec[:], ps_o[32:33, :])
                    ps_b = pbc.tile([D, EL], F32, tag="bc", name="ps_b")
                    nc.tensor.matmul(ps_b[:], ones_f[:], rec[:], start=True, stop=True)
                    bc_sb = sb.tile([D, EL], F32, tag="bcs", name="bc_sb")
                    nc.vector.tensor_copy(bc_sb[:], ps_b[:])
                    nc.vector.tensor_mul(oT[hp:hp + D, hf, :], ps_o[0:32, :], bc_sb[:])

                # I. out-proj + residual: t_ij = attn + 2r (out_b folded into up_b)
                t_bf = sb.tile([P, EC, H], BF16, name="t_bf")
                for m in range(EC):
                    ps = pmm.tile([P, EL], F32, tag="mm", name="ps_a")
                    for k in range(HK):
                        nc.tensor.matmul(
                            ps[:, :H], oT[:, k, m * P:(m + 1) * P], outw_sb[:, t, k, :],
                            start=(k == 0), stop=(k == HK - 1),
                        )
                    nc.vector.scalar_tensor_tensor(
                        t_bf[:, m, :], r_nat[:, m, :], 2.0, ps[:, :H],
                        op0=ALU.mult, op1=ALU.add,
                    )

                # J. t^T
                tT = sb.tile([P, HK, EL], BF16, name="tT")
                for c in range(EC):
                    for hf in range(HK):
                        transpose_128(
                            tT[:, hf, c * P:(c + 1) * P],
                            t_bf[:, c, hf * P:(hf + 1) * P],
                        )

                # K. up-proj + LN2 + gelu -> next h (or output)
                last = t == L - 1
                if not last:
                    h_nat_new = sb.tile([P, EC, H], BF16, name="h_nat")
                for m in range(EC):
                    ps = pmm.tile([P, EL], F32, tag="mm", name="ps_u")
                    for k in range(HK):
                        nc.tensor.matmul(
                            ps[:, :H], tT[:, k, m * P:(m + 1) * P], upw_sb[:, t, k, :],
                            start=(k == 0), stop=(k == HK - 1),
                        )
                    u32 = sb.tile([P, H], F32, tag="u32", name="u32")
                    nc.vector.tensor_add(u32[:], ps[:, :H], upb2_bc[:, t, :])
                    uln = sb.tile([P, H], F32, tag="uln", name="uln")
                    layer_norm_apply(
                        uln[:], u32[:], ln2g_bc[:, t, :], ln2b_bc[:, t, :], "xln",
                    )
                    if last:
                        hf32 = sb.tile([P, H], F32, tag="hf32", name="hf32")
                        nc.scalar.activation(hf32[:], uln[:], AF.Gelu)
                        nc.sync.dma_start(hout[m * P:(m + 1) * P, :], hf32[:])
                    else:
                        nc.scalar.activation(h_nat_new[:, m, :], uln[:], AF.Gelu)
                if not last:
                    h_nat = h_nat_new

    nc.compile()
    return nc


_NC_CACHE = None


def _get_nc():
    global _NC_CACHE
    if _NC_CACHE is None:
        _NC_CACHE = _build()
    return _NC_CACHE


def _prepare_in_maps(inputs):
    ei = np.asarray(inputs["edge_index"])
    bond = np.asarray(inputs["bond_features"], dtype=np.float32)
    W_emb = np.asarray(inputs["W_emb"], dtype=np.float32)
    b_emb = np.asarray(inputs["b_emb"], dtype=np.float32)
    W_h = np.asarray(inputs["W_h"], dtype=np.float32)
    b_h = np.asarray(inputs["b_h"], dtype=np.float32)
    ln1_g = np.asarray(inputs["ln1_g"], dtype=np.float32)
    ln1_b = np.asarray(inputs["ln1_b"], dtype=np.float32)
    in_w = np.asarray(inputs["in_w"], dtype=np.float32)
    in_b = np.asarray(inputs["in_b"], dtype=np.float32)
    out_w = np.asarray(inputs["out_w"], dtype=np.float32)
    out_b = np.asarray(inputs["out_b"], dtype=np.float32)
    up_w = np.asarray(inputs["up_w"], dtype=np.float32)
    up_b = np.asarray(inputs["up_b"], dtype=np.float32)
    ln2_g = np.asarray(inputs["ln2_g"], dtype=np.float32)
    ln2_b = np.asarray(inputs["ln2_b"], dtype=np.float32)

    tgt = ei[1].astype(np.int64)
    deg = np.zeros(NN, np.float32)
    np.add.at(deg, tgt, 1.0)
    deg_tgt = deg[tgt]  # [E]

    # scale q columns by 1/sqrt(d)
    sc = 1.0 / np.sqrt(np.float32(D))
    in_w_s = in_w.copy()
    in_w_s[:, :, :H] *= sc
    in_b_s = in_b.copy()
    in_b_s[:, :H] *= sc

    shared = {
        "wemb": W_emb.astype(BFNP),
        "bemb": b_emb.reshape(HK, P).T.copy(),
        "wh": W_h.reshape(HK, P, H).transpose(1, 0, 2).astype(BFNP),
        "bh": b_h.reshape(HK, P).T.copy(),
        "inw": in_w_s.reshape(L, HK, P, 3 * H).transpose(2, 0, 1, 3).astype(BFNP),
        "inb": in_b_s.reshape(L, M6, P).transpose(2, 0, 1).copy(),
        "outw": out_w.reshape(L, HK, P, H).transpose(2, 0, 1, 3).astype(BFNP),
        "upw": up_w.reshape(L, HK, P, H).transpose(2, 0, 1, 3).astype(BFNP),
        "upb2": (up_b + np.einsum("lh,lho->lo", out_b, up_w)).astype(np.float32),
        "ln1g": ln1_g, "ln1b": ln1_b, "ln2g": ln2_g, "ln2b": ln2_b,
    }
    shared = {k: np.ascontiguousarray(v) for k, v in shared.items()}

    in_maps = []
    for c in range(NC):
        sl = slice(c * EL, (c + 1) * EL)
        tl = tgt[sl]
        dl = deg_tgt[sl]
        A = np.zeros((EL, NN), np.float32)
        A[np.arange(EL), tl] = 1.0
        B = np.zeros((NT + EC) * P, dtype=np.float32)
        B = np.zeros(((NT + EC) * P, EL), np.float32)
        B[tl, np.arange(EL)] = 1.0
        B[NN + np.arange(EL), np.arange(EL)] = -dl
        m = {
            "bondT": np.ascontiguousarray(bond[sl].T.astype(BFNP)),
            "Amat": np.ascontiguousarray(
                A.reshape(EC, P, NN).transpose(1, 0, 2).astype(BFNP)
            ),
            "Bmat": np.ascontiguousarray(
                B.reshape(JB, P, EL).transpose(1, 0, 2).astype(BFNP)
            ),
        }
        m.update(shared)
        in_maps.append(m)
    return in_maps


def kernel(**inputs):
    nc = _get_nc()
    in_maps = _prepare_in_maps(inputs)
    res = run_bass_kernel_spmd(nc, in_maps, core_ids=list(range(NC)))
    out = np.concatenate(
        [np.asarray(res.results[c]["hout"]) for c in range(NC)], axis=0
    )
    return out.astype(np.float32)


# revision 5
# speedup vs baseline: 1.0969x; 1.0969x over previous
"""BondMessagePassing kernel for 8 Trainium2 NeuronCores.

Edge-sharded data parallelism: 512 edges per core. Per layer:
  - node segment-sum via matmul with host-built one-hot + AllReduce
  - gather + residual term via one fused matmul (B = [A^T; -diag(deg)])
  - full-sequence MHA over 4096 edges: each core computes its 512 query
    rows against the AllGathered K/V of all cores
Linears run in transposed-activation layout so weights are natural lhsT;
PE transposes switch layouts where LayerNorm/segment ops need row layout.
"""

import numpy as np
import ml_dtypes

import concourse.bass as bass
import concourse.tile as tile
import concourse.mybir as mybir
from concourse import bacc
from concourse.bass_utils import run_bass_kernel_spmd
from concourse.masks import make_identity

F32 = mybir.dt.float32
BF16 = mybir.dt.bfloat16
AF = mybir.ActivationFunctionType
ALU = mybir.AluOpType
BFNP = ml_dtypes.bfloat16

NC = 8          # cores
P = 128         # partitions
NN = 1024       # nodes
E = 4096        # edges
EL = E // NC    # edges per core (512)
H = 256         # hidden
BD = 64         # bond dim
NH = 8          # heads
D = H // NH     # head dim (32)
L = 3           # layers
HK = H // P     # 2  K-chunks per 256
EC = EL // P    # 4  edge chunks per core
NT = NN // P    # 8  node tiles
KT = E // P     # 32 k-tiles (global edges)
M6 = 3 * H // P  # 6 qkv out tiles
JB = NT + EC    # 12 K-chunks of the fused r matmul
AGW = 1024 + EC * NH * 33  # 2080 allgather row width (K^T 1024 + V_aug 1056)


def _build():
    nc = bacc.Bacc(None, target_bir_lowering=False, num_devices=NC)

    di = {}
    def din(name, shape, dtype):
        di[name] = nc.dram_tensor(name, shape, dtype, kind="ExternalInput")
        return di[name]

    din("bondT", [BD, EL], BF16)
    din("Amat", [P, EC, NN], BF16)
    din("Bmat", [P, JB, EL], BF16)
    din("wemb", [BD, H], BF16)
    din("bemb", [P, HK], F32)
    din("wh", [P, HK, H], BF16)
    din("bh", [P, HK], F32)
    din("inw", [P, L, HK, 3 * H], BF16)
    din("inb", [P, L, M6], F32)
    din("outw", [P, L, HK, H], BF16)
    din("upw", [P, L, HK, H], BF16)
    din("upb2", [L, H], F32)
    din("ln1g", [L, H], F32)
    din("ln1b", [L, H], F32)
    din("ln2g", [L, H], F32)
    din("ln2b", [L, H], F32)
    hout = nc.dram_tensor("hout", [EL, H], F32, kind="ExternalOutput")

    rg = [list(range(NC))]

    with tile.TileContext(nc) as tc:
        with (
            tc.tile_pool(name="const", bufs=1) as const,
            tc.tile_pool(name="sb", bufs=2) as sb,
            tc.tile_pool(name="kv", bufs=1) as kv,
            tc.tile_pool(name="ptp", bufs=4) as ptp,
            tc.tile_pool(name="pmm2", bufs=2, space="PSUM") as pmm2,
            tc.tile_pool(name="pacc", bufs=2, space="PSUM") as pacc,
            tc.tile_pool(name="paux", bufs=2, space="PSUM") as paux,
            tc.tile_pool(name="dram", bufs=1, space="DRAM") as dram,
        ):
            # ---- CC warmup: tiny AllReduce overlapping the embedding ----
            warm_in = dram.tile([P, 4], F32, name="warm_in")
            warm_out = dram.tile([P, 4], F32, addr_space="Shared", name="warm_out")
            wz = const.tile([P, 4], F32, name="wz")
            nc.vector.memset(wz[:], 0.0)
            nc.sync.dma_start(warm_in[:], wz[:])
            nc.gpsimd.collective_compute(
                "AllReduce", ALU.add, replica_groups=rg,
                ins=[warm_in[:]], outs=[warm_out[:]],
            )

            # ---- load constants ----
            bondT_sb = const.tile([BD, EL], BF16)
            nc.sync.dma_start(bondT_sb[:], di["bondT"][:])
            A_sb = const.tile([P, EC, NN], BF16)
            nc.sync.dma_start(A_sb[:], di["Amat"][:])
            B_sb = const.tile([P, JB, EL], BF16)
            nc.sync.dma_start(B_sb[:], di["Bmat"][:])
            wemb_sb = const.tile([BD, H], BF16)
            nc.sync.dma_start(wemb_sb[:], di["wemb"][:])
            bemb_sb = const.tile([P, HK], F32)
            nc.sync.dma_start(bemb_sb[:], di["bemb"][:])
            wh_sb = const.tile([P, HK, H], BF16)
            nc.sync.dma_start(wh_sb[:], di["wh"][:])
            bh_sb = const.tile([P, HK], F32)
            nc.sync.dma_start(bh_sb[:], di["bh"][:])
            inw_sb = const.tile([P, L, HK, 3 * H], BF16)
            nc.sync.dma_start(inw_sb[:], di["inw"][:])
            inb_sb = const.tile([P, L, M6], F32)
            nc.sync.dma_start(inb_sb[:], di["inb"][:])
            outw_sb = const.tile([P, L, HK, H], BF16)
            nc.sync.dma_start(outw_sb[:], di["outw"][:])
            upw_sb = const.tile([P, L, HK, H], BF16)
            nc.sync.dma_start(upw_sb[:], di["upw"][:])

            def bcast_load(name):
                t = const.tile([P, L, H], F32, name=f"{name}_bc")
                src = di[name][:]
                bap = bass.AP(
                    tensor=src.tensor,
                    offset=src.offset,
                    ap=[[0, P]] + [list(x) for x in src.ap],
                )
                nc.sync.dma_start(t[:], bap)
                return t

            upb2_bc = bcast_load("upb2")
            ln1g_bc = bcast_load("ln1g")
            ln1b_bc = bcast_load("ln1b")
            ln2g_bc = bcast_load("ln2g")
            ln2b_bc = bcast_load("ln2b")

            ident_bf = const.tile([P, P], BF16)
            make_identity(nc, ident_bf[:])
            ones_f = const.tile([1, D], F32)
            nc.vector.memset(ones_f[:], 1.0)
            eps_sb = const.tile([P, 1], F32)
            nc.vector.memset(eps_sb[:], 1e-5)

            def transpose_128(dst_ap, src_ap):
                pst = paux.tile([P, P], BF16, tag="aux", name="pst")
                nc.tensor.transpose(pst[:], src_ap, ident_bf[:])
                nc.vector.tensor_copy(dst_ap, pst[:])

            # ---- embedding: h = gelu(bond @ W_emb + b_emb) @ W_h + b_h ----
            g1 = sb.tile([P, HK, EL], BF16, name="g1")
            for m in range(HK):
                ps = paux.tile([P, EL], F32, tag="aux", name="ps_e")
                nc.tensor.matmul(
                    ps[:], wemb_sb[:, m * P:(m + 1) * P], bondT_sb[:],
                    start=True, stop=True,
                )
                nc.scalar.activation(
                    g1[:, m, :], ps[:], AF.Gelu, bias=bemb_sb[:, m:m + 1]
                )
            hT = sb.tile([P, HK, EL], BF16, name="hT")
            for m in range(HK):
                ps = paux.tile([P, EL], F32, tag="aux", name="ps_h")
                for k in range(HK):
                    nc.tensor.matmul(
                        ps[:], wh_sb[:, k, m * P:(m + 1) * P], g1[:, k, :],
                        start=(k == 0), stop=(k == HK - 1),
                    )
                nc.vector.tensor_scalar_add(hT[:, m, :], ps[:], bh_sb[:, m:m + 1])
            h_nat = sb.tile([P, EC, H], BF16, name="h_nat")
            for m in range(HK):
                for c in range(EC):
                    transpose_128(
                        h_nat[:, c, m * P:(m + 1) * P],
                        hT[:, m, c * P:(c + 1) * P],
                    )

            # ---- layers ----
            for t in range(L):
                # A. partial segment-sum over local edges, AllReduce (bf16)
                ar_in = dram.tile([NN, H], BF16, name=f"ar_in{t}")
                ar_out = dram.tile([NN, H], BF16, addr_space="Shared", name=f"ar_out{t}")
                for i in range(NT):
                    ps = paux.tile([P, EL], F32, tag="aux", name="ps_s")
                    for c in range(EC):
                        nc.tensor.matmul(
                            ps[:, :H], A_sb[:, c, i * P:(i + 1) * P], h_nat[:, c, :],
                            start=(c == 0), stop=(c == EC - 1),
                        )
                    s16 = sb.tile([P, H], BF16, tag="s16", name="s16")
                    nc.vector.tensor_copy(s16[:], ps[:, :H])
                    nc.sync.dma_start(ar_in[i * P:(i + 1) * P, :], s16[:])
                nc.gpsimd.collective_compute(
                    "AllReduce", ALU.add, replica_groups=rg,
                    ins=[ar_in[:]], outs=[ar_out[:]],
                )
                s_bf = sb.tile([P, NT, H], BF16, name="s_bf")
                nc.sync.dma_start(
                    s_bf[:],
                    ar_out[:].rearrange("(i p) h -> p i h", p=P),
                )

                # B. r = S[tgt] - deg[tgt]*h  (fused matmul), keep f32
                r_nat = sb.tile([P, EC, H], F32, name="r_nat")
                mv4 = sb.tile([P, EC, 2], F32, name="mv4")
                for m in range(EC):
                    ps = paux.tile([P, EL], F32, tag="aux", name="ps_r")
                    for j in range(JB):
                        rhs = s_bf[:, j, :] if j < NT else h_nat[:, j - NT, :]
                        nc.tensor.matmul(
                            ps[:, :H], B_sb[:, j, m * P:(m + 1) * P], rhs,
                            start=(j == 0), stop=(j == JB - 1),
                        )
                    nc.vector.tensor_copy(r_nat[:, m, :], ps[:, :H])
                    stats = sb.tile([P, 6], F32, tag="stats", name="stats")
                    nc.vector.bn_stats(stats[:], ps[:, :H])
                    nc.vector.bn_aggr(mv4[:, m, :], stats[:])
                # C. LN1 -> xn (bf16): batched rstd then apply
                rstd4 = sb.tile([P, EC], F32, name="rstd4")
                nc.scalar.activation(rstd4[:], mv4[:, :, 1], AF.Sqrt, bias=eps_sb[:])
                nc.vector.reciprocal(rstd4[:], rstd4[:])
                xn_bf = sb.tile([P, EC, H], BF16, name="xn_bf")
                for m in range(EC):
                    xc = sb.tile([P, H], F32, tag="xn32", name="xn32")
                    nc.vector.tensor_scalar(
                        xc[:], r_nat[:, m, :], mv4[:, m, 0:1], rstd4[:, m:m + 1],
                        op0=ALU.subtract, op1=ALU.mult,
                    )
                    nc.vector.tensor_mul(xc[:], xc[:], ln1g_bc[:, t, :])
                    nc.vector.tensor_add(xn_bf[:, m, :], xc[:], ln1b_bc[:, t, :])

                # D. xn^T
                xnT = sb.tile([P, HK, EL], BF16, name="xnT")
                for c in range(EC):
                    for hf in range(HK):
                        transpose_128(
                            xnT[:, hf, c * P:(c + 1) * P],
                            xn_bf[:, c, hf * P:(hf + 1) * P],
                        )

                # E. in-proj: qkv^T = in_w^T @ xn^T + in_b
                QT = sb.tile([P, HK, EL], BF16, name="QT")
                KTl = sb.tile([P, HK, EL], BF16, name="KTl")
                VTl = sb.tile([P, HK, EL], BF16, name="VTl")
                dests = [(QT, 0), (QT, 1), (KTl, 0), (KTl, 1), (VTl, 0), (VTl, 1)]
                for m in range(M6):
                    ps = paux.tile([P, EL], F32, tag="aux", name="ps_q")
                    for k in range(HK):
                        nc.tensor.matmul(
                            ps[:], inw_sb[:, t, k, m * P:(m + 1) * P], xnT[:, k, :],
                            start=(k == 0), stop=(k == HK - 1),
                        )
                    dt_, idx = dests[m]
                    nc.vector.tensor_scalar_add(
                        dt_[:, idx, :], ps[:], inb_sb[:, t, m:m + 1]
                    )

                # F. v natural + ones column (V_aug), local shard
                vnat = sb.tile([P, EC, NH, 33], BF16, name="vnat")
                for hf in range(HK):
                    for c in range(EC):
                        pst = paux.tile([P, P], BF16, tag="aux", name="pst_v")
                        nc.tensor.transpose(
                            pst[:], VTl[:, hf, c * P:(c + 1) * P], ident_bf[:]
                        )
                        nc.vector.tensor_copy(
                            vnat[:, c, hf * 4:(hf + 1) * 4, 0:32],
                            pst[:].rearrange("p (a b) -> p a b", a=4),
                        )
                nc.vector.memset(vnat[:, :, :, 32:33], 1.0)

                # G. AllGather K^T and V_aug
                ag_in = dram.tile([P, AGW], BF16, name=f"ag_in{t}")
                ag_out = dram.tile(
                    [P * NC, AGW], BF16, addr_space="Shared", name=f"ag_out{t}"
                )
                nc.sync.dma_start(
                    ag_in[:, 0:1024].rearrange("p (a b) -> p a b", a=HK), KTl[:]
                )
                nc.sync.dma_start(
                    ag_in[:, 1024:AGW].rearrange(
                        "p (a b c) -> p a b c", a=EC, b=NH
                    ),
                    vnat[:],
                )
                nc.gpsimd.collective_compute(
                    "AllGather", ALU.bypass, replica_groups=rg,
                    ins=[ag_in[:]], outs=[ag_out[:]],
                )
                KT_all = kv.tile([P, NC, HK, EL], BF16, name="KT_all")
                V_all = kv.tile([P, NC, EC, NH, 33], BF16, name="V_all")
                for s in range(NC):
                    nc.sync.dma_start(
                        KT_all[:, s, :, :],
                        ag_out[s * P:(s + 1) * P, 0:1024].rearrange(
                            "p (a b) -> p a b", a=HK
                        ),
                    )
                    nc.sync.dma_start(
                        V_all[:, s, :, :, :],
                        ag_out[s * P:(s + 1) * P, 1024:AGW].rearrange(
                            "p (a b c) -> p a b c", a=EC, b=NH
                        ),
                    )

                # H. attention (per head): scores^T 2-bank tiles, batched exp,
                #    PV with ones row for the softmax denominator
                oT = sb.tile([P, HK, EL], BF16, name="oT")
                for h in range(NH):
                    hp = (h % 4) * D
                    hf = h // 4
                    ps_o = pacc.tile([33, EL], F32, tag="acc", name="ps_o")
                    for kt2 in range(0, KT, 2):
                        ps2 = pmm2.tile([P, 2, EL], F32, tag="mm", name="ps2")
                        for j in range(2):
                            s, c = divmod(kt2 + j, EC)
                            nc.tensor.matmul(
                                ps2[:, j, :],
                                KT_all[hp:hp + D, s, hf, c * P:(c + 1) * P],
                                QT[hp:hp + D, hf, :],
                                start=True, stop=True,
                                tile_position=(hp, 0),
                            )
                        pt2 = ptp.tile([P, 2, EL], BF16, tag="pt", name="pt")
                        nc.scalar.activation(pt2[:], ps2[:], AF.Exp)
                        for j in range(2):
                            s, c = divmod(kt2 + j, EC)
                            nc.tensor.matmul(
                                ps_o[:], V_all[:, s, c, h, 0:33], pt2[:, j, :],
                                start=(kt2 + j == 0), stop=(kt2 + j == KT - 1),
                            )
                    den = sb.tile([1, EL], F32, tag="den", name="den")
                    nc.vector.tensor_copy(den[:], ps_o[32:33, :])
                    ps_b = paux.tile([D, EL], F32, tag="aux", name="ps_b")
                    nc.tensor.matmul(ps_b[:], ones_f[:], den[:], start=True, stop=True)
                    rec32 = sb.tile([D, EL], F32, tag="rec32", name="rec32")
                    nc.vector.reciprocal(rec32[:], ps_b[:])
                    nc.vector.tensor_mul(oT[hp:hp + D, hf, :], ps_o[0:32, :], rec32[:])

                # I. out-proj + residual: t_ij = attn + 2r (out_b folded into up_b)
                t_bf = sb.tile([P, EC, H], BF16, name="t_bf")
                for m in range(EC):
                    ps = paux.tile([P, EL], F32, tag="aux", name="ps_a")
                    for k in range(HK):
                        nc.tensor.matmul(
                            ps[:, :H], oT[:, k, m * P:(m + 1) * P], outw_sb[:, t, k, :],
                            start=(k == 0), stop=(k == HK - 1),
                        )
                    nc.vector.scalar_tensor_tensor(
                        t_bf[:, m, :], r_nat[:, m, :], 2.0, ps[:, :H],
                        op0=ALU.mult, op1=ALU.add,
                    )

                # J. t^T
                tT = sb.tile([P, HK, EL], BF16, name="tT")
                for c in range(EC):
                    for hf in range(HK):
                        transpose_128(
                            tT[:, hf, c * P:(c + 1) * P],
                            t_bf[:, c, hf * P:(hf + 1) * P],
                        )

                # K. up-proj + LN2 + gelu -> next h (or output)
                last = t == L - 1
                if not last:
                    h_nat_new = sb.tile([P, EC, H], BF16, name="h_nat")
                u4 = sb.tile([P, EC, H], F32, name="u4")
                mv4b = sb.tile([P, EC, 2], F32, name="mv4b")
                for m in range(EC):
                    ps = paux.tile([P, EL], F32, tag="aux", name="ps_u")
                    for k in range(HK):
                        nc.tensor.matmul(
                            ps[:, :H], tT[:, k, m * P:(m + 1) * P], upw_sb[:, t, k, :],
                            start=(k == 0), stop=(k == HK - 1),
                        )
                    nc.vector.tensor_add(u4[:, m, :], ps[:, :H], upb2_bc[:, t, :])
                    stats = sb.tile([P, 6], F32, tag="stats", name="stats")
                    nc.vector.bn_stats(stats[:], u4[:, m, :])
                    nc.vector.bn_aggr(mv4b[:, m, :], stats[:])
                rstd4b = sb.tile([P, EC], F32, name="rstd4b")
                nc.scalar.activation(rstd4b[:], mv4b[:, :, 1], AF.Sqrt, bias=eps_sb[:])
                nc.vector.reciprocal(rstd4b[:], rstd4b[:])
                for m in range(EC):
                    xc = sb.tile([P, H], F32, tag="xln", name="xln")
                    nc.vector.tensor_scalar(
                        xc[:], u4[:, m, :], mv4b[:, m, 0:1], rstd4b[:, m:m + 1],
                        op0=ALU.subtract, op1=ALU.mult,
                    )
                    nc.vector.tensor_mul(xc[:], xc[:], ln2g_bc[:, t, :])
                    uln = sb.tile([P, H], F32, tag="uln", name="uln")
                    nc.vector.tensor_add(uln[:], xc[:], ln2b_bc[:, t, :])
                    if last:
                        hf32 = sb.tile([P, H], F32, tag="hf32", name="hf32")
                        nc.scalar.activation(hf32[:], uln[:], AF.Gelu)
                        nc.sync.dma_start(hout[m * P:(m + 1) * P, :], hf32[:])
                    else:
                        nc.scalar.activation(h_nat_new[:, m, :], uln[:], AF.Gelu)
                if not last:
                    h_nat = h_nat_new

    nc.compile()
    return nc


_NC_CACHE = None


def _get_nc():
    global _NC_CACHE
    if _NC_CACHE is None:
        _NC_CACHE = _build()
    return _NC_CACHE


def _prepare_in_maps(inputs):
    ei = np.asarray(inputs["edge_index"])
    bond = np.asarray(inputs["bond_features"], dtype=np.float32)
    W_emb = np.asarray(inputs["W_emb"], dtype=np.float32)
    b_emb = np.asarray(inputs["b_emb"], dtype=np.float32)
    W_h = np.asarray(inputs["W_h"], dtype=np.float32)
    b_h = np.asarray(inputs["b_h"], dtype=np.float32)
    ln1_g = np.asarray(inputs["ln1_g"], dtype=np.float32)
    ln1_b = np.asarray(inputs["ln1_b"], dtype=np.float32)
    in_w = np.asarray(inputs["in_w"], dtype=np.float32)
    in_b = np.asarray(inputs["in_b"], dtype=np.float32)
    out_w = np.asarray(inputs["out_w"], dtype=np.float32)
    out_b = np.asarray(inputs["out_b"], dtype=np.float32)
    up_w = np.asarray(inputs["up_w"], dtype=np.float32)
    up_b = np.asarray(inputs["up_b"], dtype=np.float32)
    ln2_g = np.asarray(inputs["ln2_g"], dtype=np.float32)
    ln2_b = np.asarray(inputs["ln2_b"], dtype=np.float32)

    tgt = ei[1].astype(np.int64)
    deg = np.zeros(NN, np.float32)
    np.add.at(deg, tgt, 1.0)
    deg_tgt = deg[tgt]  # [E]

    # scale q columns by 1/sqrt(d)
    sc = 1.0 / np.sqrt(np.float32(D))
    in_w_s = in_w.copy()
    in_w_s[:, :, :H] *= sc
    in_b_s = in_b.copy()
    in_b_s[:, :H] *= sc

    shared = {
        "wemb": W_emb.astype(BFNP),
        "bemb": b_emb.reshape(HK, P).T.copy(),
        "wh": W_h.reshape(HK, P, H).transpose(1, 0, 2).astype(BFNP),
        "bh": b_h.reshape(HK, P).T.copy(),
        "inw": in_w_s.reshape(L, HK, P, 3 * H).transpose(2, 0, 1, 3).astype(BFNP),
        "inb": in_b_s.reshape(L, M6, P).transpose(2, 0, 1).copy(),
        "outw": out_w.reshape(L, HK, P, H).transpose(2, 0, 1, 3).astype(BFNP),
        "upw": up_w.reshape(L, HK, P, H).transpose(2, 0, 1, 3).astype(BFNP),
        "upb2": (up_b + np.einsum("lh,lho->lo", out_b, up_w)).astype(np.float32),
        "ln1g": ln1_g, "ln1b": ln1_b, "ln2g": ln2_g, "ln2b": ln2_b,
    }
    shared = {k: np.ascontiguousarray(v) for k, v in shared.items()}

    in_maps = []
    for c in range(NC):
        sl = slice(c * EL, (c + 1) * EL)
        tl = tgt[sl]
        dl = deg_tgt[sl]
        A = np.zeros((EL, NN), np.float32)
        A[np.arange(EL), tl] = 1.0
        B = np.zeros(((NT + EC) * P, EL), np.float32)
        B[tl, np.arange(EL)] = 1.0
        B[NN + np.arange(EL), np.arange(EL)] = -dl
        m = {
            "bondT": np.ascontiguousarray(bond[sl].T.astype(BFNP)),
            "Amat": np.ascontiguousarray(
                A.reshape(EC, P, NN).transpose(1, 0, 2).astype(BFNP)
            ),
            "Bmat": np.ascontiguousarray(
                B.reshape(JB, P, EL).transpose(1, 0, 2).astype(BFNP)
            ),
        }
        m.update(shared)
        in_maps.append(m)
    return in_maps


def kernel(**inputs):
    nc = _get_nc()
    in_maps = _prepare_in_maps(inputs)
    res = run_bass_kernel_spmd(nc, in_maps, core_ids=list(range(NC)))
    out = np.concatenate(
        [np.asarray(res.results[c]["hout"]) for c in range(NC)], axis=0
    )
    return out.astype(np.float32)


# revision 7
# speedup vs baseline: 1.3896x; 1.2668x over previous
"""BondMessagePassing kernel for 8 Trainium2 NeuronCores.

Edge-sharded data parallelism: 512 edges per core. Per layer:
  - node segment-sum via matmul with host-built one-hot + AllReduce
  - gather + residual term via one fused matmul (B = [A^T; -diag(deg)])
  - full-sequence MHA over 4096 edges: each core computes its 512 query
    rows against the AllGathered K/V of all cores
Linears run in transposed-activation layout so weights are natural lhsT;
PE transposes switch layouts where LayerNorm/segment ops need row layout.
"""

import numpy as np
import ml_dtypes

import concourse.bass as bass
import concourse.tile as tile
import concourse.mybir as mybir
from concourse import bacc
from concourse.bass_utils import run_bass_kernel_spmd
from concourse.masks import make_identity

F32 = mybir.dt.float32
BF16 = mybir.dt.bfloat16
AF = mybir.ActivationFunctionType
ALU = mybir.AluOpType
BFNP = ml_dtypes.bfloat16

NC = 8          # cores
P = 128         # partitions
NN = 1024       # nodes
E = 4096        # edges
EL = E // NC    # edges per core (512)
H = 256         # hidden
BD = 64         # bond dim
NH = 8          # heads
D = H // NH     # head dim (32)
L = 3           # layers
HK = H // P     # 2  K-chunks per 256
EC = EL // P    # 4  edge chunks per core
NT = NN // P    # 8  node tiles
KT = E // P     # 32 k-tiles (global edges)
M6 = 3 * H // P  # 6 qkv out tiles
JB = NT + EC    # 12 K-chunks of the fused r matmul
AGW = 1024 + EC * NH * 33  # 2080 allgather row width (K^T 1024 + V_aug 1056)


def _build():
    nc = bacc.Bacc(None, target_bir_lowering=False, num_devices=NC)

    di = {}
    def din(name, shape, dtype):
        di[name] = nc.dram_tensor(name, shape, dtype, kind="ExternalInput")
        return di[name]

    din("bondT", [BD, EL], BF16)
    din("Amat", [P, EC, NN], BF16)
    din("Bmat", [P, JB, EL], BF16)
    din("wemb", [BD, H], BF16)
    din("bemb", [P, HK], F32)
    din("wh", [P, HK, H], BF16)
    din("bh", [P, HK], F32)
    din("inw", [P, L, HK, 3 * H], BF16)
    din("inb", [P, L, M6], F32)
    din("outw", [P, L, HK, H], BF16)
    din("upw", [P, L, HK, H], BF16)
    din("upb2", [L, H], F32)
    din("ln1g", [L, H], F32)
    din("ln1b", [L, H], F32)
    din("ln2g", [L, H], F32)
    din("ln2b", [L, H], F32)
    hout = nc.dram_tensor("hout", [EL, H], F32, kind="ExternalOutput")

    rg = [list(range(NC))]

    with tile.TileContext(nc) as tc:
        with (
            tc.tile_pool(name="const", bufs=1) as const,
            tc.tile_pool(name="sb", bufs=2) as sb,
            tc.tile_pool(name="kv", bufs=1) as kv,
            tc.tile_pool(name="ptp", bufs=4) as ptp,
            tc.tile_pool(name="pmm2", bufs=2, space="PSUM") as pmm2,
            tc.tile_pool(name="pacc", bufs=2, space="PSUM") as pacc,
            tc.tile_pool(name="paux", bufs=2, space="PSUM") as paux,
            tc.tile_pool(name="dram", bufs=1, space="DRAM") as dram,
        ):
            # ---- load constants ----
            bondT_sb = const.tile([BD, EL], BF16)
            nc.sync.dma_start(bondT_sb[:], di["bondT"][:])
            A_sb = const.tile([P, EC, NN], BF16)
            nc.sync.dma_start(A_sb[:], di["Amat"][:])
            B_sb = const.tile([P, JB, EL], BF16)
            nc.sync.dma_start(B_sb[:], di["Bmat"][:])
            wemb_sb = const.tile([BD, H], BF16)
            nc.sync.dma_start(wemb_sb[:], di["wemb"][:])
            bemb_sb = const.tile([P, HK], F32)
            nc.sync.dma_start(bemb_sb[:], di["bemb"][:])
            wh_sb = const.tile([P, HK, H], BF16)
            nc.sync.dma_start(wh_sb[:], di["wh"][:])
            bh_sb = const.tile([P, HK], F32)
            nc.sync.dma_start(bh_sb[:], di["bh"][:])
            inw_sb = const.tile([P, L, HK, 3 * H], BF16)
            nc.sync.dma_start(inw_sb[:], di["inw"][:])
            inb_sb = const.tile([P, L, M6], F32)
            nc.sync.dma_start(inb_sb[:], di["inb"][:])
            outw_sb = const.tile([P, L, HK, H], BF16)
            nc.sync.dma_start(outw_sb[:], di["outw"][:])
            upw_sb = const.tile([P, L, HK, H], BF16)
            nc.sync.dma_start(upw_sb[:], di["upw"][:])

            def bcast_load(name):
                t = const.tile([P, L, H], F32, name=f"{name}_bc")
                src = di[name][:]
                bap = bass.AP(
                    tensor=src.tensor,
                    offset=src.offset,
                    ap=[[0, P]] + [list(x) for x in src.ap],
                )
                nc.sync.dma_start(t[:], bap)
                return t

            upb2_bc = bcast_load("upb2")
            ln1g_bc = bcast_load("ln1g")
            ln1b_bc = bcast_load("ln1b")
            ln2g_bc = bcast_load("ln2g")
            ln2b_bc = bcast_load("ln2b")

            ident_bf = const.tile([P, P], BF16)
            make_identity(nc, ident_bf[:])
            ones_f = const.tile([1, D], F32)
            nc.vector.memset(ones_f[:], 1.0)
            eps_sb = const.tile([P, 1], F32)
            nc.vector.memset(eps_sb[:], 1e-5)

            def transpose_128(dst_ap, src_ap):
                pst = paux.tile([P, P], BF16, tag="aux", name="pst")
                nc.tensor.transpose(pst[:], src_ap, ident_bf[:])
                nc.vector.tensor_copy(dst_ap, pst[:])

            # ---- embedding: h = gelu(bond @ W_emb + b_emb) @ W_h + b_h ----
            g1 = sb.tile([P, HK, EL], BF16, name="g1")
            for m in range(HK):
                ps = paux.tile([P, EL], F32, tag="aux", name="ps_e")
                nc.tensor.matmul(
                    ps[:], wemb_sb[:, m * P:(m + 1) * P], bondT_sb[:],
                    start=True, stop=True,
                )
                nc.scalar.activation(
                    g1[:, m, :], ps[:], AF.Gelu, bias=bemb_sb[:, m:m + 1]
                )
            hT = sb.tile([P, HK, EL], BF16, name="hT")
            for m in range(HK):
                ps = paux.tile([P, EL], F32, tag="aux", name="ps_h")
                for k in range(HK):
                    nc.tensor.matmul(
                        ps[:], wh_sb[:, k, m * P:(m + 1) * P], g1[:, k, :],
                        start=(k == 0), stop=(k == HK - 1),
                    )
                nc.vector.tensor_scalar_add(hT[:, m, :], ps[:], bh_sb[:, m:m + 1])
            h_nat = sb.tile([P, EC, H], BF16, name="h_nat")
            for m in range(HK):
                for c in range(EC):
                    transpose_128(
                        h_nat[:, c, m * P:(m + 1) * P],
                        hT[:, m, c * P:(c + 1) * P],
                    )

            # ---- layers ----
            for t in range(L):
                # A. partial segment-sum over local edges, AllReduce (bf16)
                ar_in = dram.tile([NN, H], BF16, name=f"ar_in{t}")
                ar_out = dram.tile([NN, H], BF16, addr_space="Shared", name=f"ar_out{t}")
                for i in range(NT):
                    ps = paux.tile([P, EL], F32, tag="aux", name="ps_s")
                    for c in range(EC):
                        nc.tensor.matmul(
                            ps[:, :H], A_sb[:, c, i * P:(i + 1) * P], h_nat[:, c, :],
                            start=(c == 0), stop=(c == EC - 1),
                        )
                    s16 = sb.tile([P, H], BF16, tag="s16", name="s16")
                    nc.vector.tensor_copy(s16[:], ps[:, :H])
                    nc.sync.dma_start(ar_in[i * P:(i + 1) * P, :], s16[:])
                nc.gpsimd.collective_compute(
                    "AllReduce", ALU.add, replica_groups=rg,
                    ins=[ar_in[:]], outs=[ar_out[:]],
                )
                s_bf = sb.tile([P, NT, H], BF16, name="s_bf")
                nc.sync.dma_start(
                    s_bf[:],
                    ar_out[:].rearrange("(i p) h -> p i h", p=P),
                )

                # B. r = S[tgt] - deg[tgt]*h  (fused matmul), keep f32
                r_nat = sb.tile([P, EC, H], F32, name="r_nat")
                mv4 = sb.tile([P, EC, 2], F32, name="mv4")
                for m in range(EC):
                    ps = paux.tile([P, EL], F32, tag="aux", name="ps_r")
                    for j in range(JB):
                        rhs = s_bf[:, j, :] if j < NT else h_nat[:, j - NT, :]
                        nc.tensor.matmul(
                            ps[:, :H], B_sb[:, j, m * P:(m + 1) * P], rhs,
                            start=(j == 0), stop=(j == JB - 1),
                        )
                    nc.vector.tensor_copy(r_nat[:, m, :], ps[:, :H])
                    stats = sb.tile([P, 6], F32, tag="stats", name="stats")
                    nc.vector.bn_stats(stats[:], ps[:, :H])
                    nc.vector.bn_aggr(mv4[:, m, :], stats[:])
                # C. LN1 -> xn (bf16): batched rstd then apply
                rstd4 = sb.tile([P, EC], F32, name="rstd4")
                nc.scalar.activation(rstd4[:], mv4[:, :, 1], AF.Sqrt, bias=eps_sb[:])
                nc.vector.reciprocal(rstd4[:], rstd4[:])
                xn_bf = sb.tile([P, EC, H], BF16, name="xn_bf")
                for m in range(EC):
                    xc = sb.tile([P, H], F32, tag="xn32", name="xn32")
                    nc.vector.tensor_scalar(
                        xc[:], r_nat[:, m, :], mv4[:, m, 0:1], rstd4[:, m:m + 1],
                        op0=ALU.subtract, op1=ALU.mult,
                    )
                    nc.vector.tensor_mul(xc[:], xc[:], ln1g_bc[:, t, :])
                    nc.vector.tensor_add(xn_bf[:, m, :], xc[:], ln1b_bc[:, t, :])

                # D. xn^T
                xnT = sb.tile([P, HK, EL], BF16, name="xnT")
                for c in range(EC):
                    for hf in range(HK):
                        transpose_128(
                            xnT[:, hf, c * P:(c + 1) * P],
                            xn_bf[:, c, hf * P:(hf + 1) * P],
                        )

                # E. in-proj: qkv^T = in_w^T @ xn^T + in_b
                QT = sb.tile([P, HK, EL], BF16, name="QT")
                KTl = sb.tile([P, HK, EL], BF16, name="KTl")
                VTl = sb.tile([P, HK, EL], BF16, name="VTl")
                dests = [(QT, 0), (QT, 1), (KTl, 0), (KTl, 1), (VTl, 0), (VTl, 1)]
                for m in range(M6):
                    ps = paux.tile([P, EL], F32, tag="aux", name="ps_q")
                    for k in range(HK):
                        nc.tensor.matmul(
                            ps[:], inw_sb[:, t, k, m * P:(m + 1) * P], xnT[:, k, :],
                            start=(k == 0), stop=(k == HK - 1),
                        )
                    dt_, idx = dests[m]
                    nc.vector.tensor_scalar_add(
                        dt_[:, idx, :], ps[:], inb_sb[:, t, m:m + 1]
                    )

                # F. v natural + ones column (V_aug), local shard
                vnat = sb.tile([P, EC, NH, 33], BF16, name="vnat")
                for hf in range(HK):
                    for c in range(EC):
                        pst = paux.tile([P, P], BF16, tag="aux", name="pst_v")
                        nc.tensor.transpose(
                            pst[:], VTl[:, hf, c * P:(c + 1) * P], ident_bf[:]
                        )
                        nc.vector.tensor_copy(
                            vnat[:, c, hf * 4:(hf + 1) * 4, 0:32],
                            pst[:].rearrange("p (a b) -> p a b", a=4),
                        )
                nc.vector.memset(vnat[:, :, :, 32:33], 1.0)

                # G. AllGather K^T and V_aug
                ag_in = dram.tile([P, AGW], BF16, name=f"ag_in{t}")
                ag_out = dram.tile(
                    [P * NC, AGW], BF16, addr_space="Shared", name=f"ag_out{t}"
                )
                nc.sync.dma_start(
                    ag_in[:, 0:1024].rearrange("p (a b) -> p a b", a=HK), KTl[:]
                )
                nc.sync.dma_start(
                    ag_in[:, 1024:AGW].rearrange(
                        "p (a b c) -> p a b c", a=EC, b=NH
                    ),
                    vnat[:],
                )
                nc.gpsimd.collective_compute(
                    "AllGather", ALU.bypass, replica_groups=rg,
                    ins=[ag_in[:]], outs=[ag_out[:]],
                )
                KT_all = kv.tile([P, NC, HK, EL], BF16, name="KT_all")
                V_all = kv.tile([P, NC, EC, NH, 33], BF16, name="V_all")
                for s in range(NC):
                    nc.sync.dma_start(
                        KT_all[:, s, :, :],
                        ag_out[s * P:(s + 1) * P, 0:1024].rearrange(
                            "p (a b) -> p a b", a=HK
                        ),
                    )
                    nc.sync.dma_start(
                        V_all[:, s, :, :, :],
                        ag_out[s * P:(s + 1) * P, 1024:AGW].rearrange(
                            "p (a b c) -> p a b c", a=EC, b=NH
                        ),
                    )

                # H. attention: pairs of heads on disjoint PE quadrants so the
                #    K=32 QK matmuls overlap in the array; scores for the pair
                #    share one 2-bank PSUM tile -> single batched exp; PV with
                #    ones row gives the softmax denominator.
                oT = sb.tile([P, HK, EL], BF16, name="oT")
                for hA, hB in ((0, 2), (1, 3), (4, 6), (5, 7)):
                    pair = (hA, hB)
                    accs = [
                        pacc.tile([33, EL], F32, tag="acc", name="ps_o")
                        for _ in range(2)
                    ]
                    for kt in range(KT):
                        s, c = divmod(kt, EC)
                        ps2 = pmm2.tile([P, 2, EL], F32, tag="mm", name="ps2")
                        for j, h in enumerate(pair):
                            hp = (h % 4) * D
                            hf = h // 4
                            nc.tensor.matmul(
                                ps2[:, j, :],
                                KT_all[hp:hp + D, s, hf, c * P:(c + 1) * P],
                                QT[hp:hp + D, hf, :],
                                start=True, stop=True,
                                tile_position=(hp, 0),
                            )
                        pt2 = ptp.tile([P, 2, EL], BF16, tag="pt", name="pt")
                        nc.scalar.activation(pt2[:], ps2[:], AF.Exp)
                        for j, h in enumerate(pair):
                            nc.tensor.matmul(
                                accs[j][:], V_all[:, s, c, h, 0:33], pt2[:, j, :],
                                start=(kt == 0), stop=(kt == KT - 1),
                            )
                    for j, h in enumerate(pair):
                        hp = (h % 4) * D
                        hf = h // 4
                        den = sb.tile([1, EL], F32, tag="den", name="den")
                        nc.vector.reciprocal(den[:], accs[j][32:33, :])
                        ps_b = paux.tile([D, EL], F32, tag="aux", name="ps_b")
                        nc.tensor.matmul(
                            ps_b[:], ones_f[:], den[:], start=True, stop=True
                        )
                        rec32 = sb.tile([D, EL], F32, tag="rec32", name="rec32")
                        nc.vector.tensor_copy(rec32[:], ps_b[:])
                        nc.vector.tensor_mul(
                            oT[hp:hp + D, hf, :], accs[j][0:32, :], rec32[:]
                        )

                # I. out-proj + residual: t_ij = attn + 2r (out_b folded into up_b)
                t_bf = sb.tile([P, EC, H], BF16, name="t_bf")
                for m in range(EC):
                    ps = paux.tile([P, EL], F32, tag="aux", name="ps_a")
                    for k in range(HK):
                        nc.tensor.matmul(
                            ps[:, :H], oT[:, k, m * P:(m + 1) * P], outw_sb[:, t, k, :],
                            start=(k == 0), stop=(k == HK - 1),
                        )
                    nc.vector.scalar_tensor_tensor(
                        t_bf[:, m, :], r_nat[:, m, :], 2.0, ps[:, :H],
                        op0=ALU.mult, op1=ALU.add,
                    )

                # J. t^T
                tT = sb.tile([P, HK, EL], BF16, name="tT")
                for c in range(EC):
                    for hf in range(HK):
                        transpose_128(
                            tT[:, hf, c * P:(c + 1) * P],
                            t_bf[:, c, hf * P:(hf + 1) * P],
                        )

                # K. up-proj + LN2 + gelu -> next h (or output)
                last = t == L - 1
                if not last:
                    h_nat_new = sb.tile([P, EC, H], BF16, name="h_nat")
                u4 = sb.tile([P, EC, H], F32, name="u4")
                mv4b = sb.tile([P, EC, 2], F32, name="mv4b")
                for m in range(EC):
                    ps = paux.tile([P, EL], F32, tag="aux", name="ps_u")
                    for k in range(HK):
                        nc.tensor.matmul(
                            ps[:, :H], tT[:, k, m * P:(m + 1) * P], upw_sb[:, t, k, :],
                            start=(k == 0), stop=(k == HK - 1),
                        )
                    nc.vector.tensor_add(u4[:, m, :], ps[:, :H], upb2_bc[:, t, :])
                    stats = sb.tile([P, 6], F32, tag="stats", name="stats")
                    nc.vector.bn_stats(stats[:], u4[:, m, :])
                    nc.vector.bn_aggr(mv4b[:, m, :], stats[:])
                rstd4b = sb.tile([P, EC], F32, name="rstd4b")
                nc.scalar.activation(rstd4b[:], mv4b[:, :, 1], AF.Sqrt, bias=eps_sb[:])
                nc.vector.reciprocal(rstd4b[:], rstd4b[:])
                for m in range(EC):
                    xc = sb.tile([P, H], F32, tag="xln", name="xln")
                    nc.vector.tensor_scalar(
                        xc[:], u4[:, m, :], mv4b[:, m, 0:1], rstd4b[:, m:m + 1],
                        op0=ALU.subtract, op1=ALU.mult,
                    )
                    nc.vector.tensor_mul(xc[:], xc[:], ln2g_bc[:, t, :])
                    uln = sb.tile([P, H], F32, tag="uln", name="uln")
                    nc.vector.tensor_add(uln[:], xc[:], ln2b_bc[:, t, :])
                    if last:
                        hf32 = sb.tile([P, H], F32, tag="hf32", name="hf32")
                        nc.scalar.activation(hf32[:], uln[:], AF.Gelu)
                        nc.sync.dma_start(hout[m * P:(m + 1) * P, :], hf32[:])
                    else:
                        nc.scalar.activation(h_nat_new[:, m, :], uln[:], AF.Gelu)
                if not last:
                    h_nat = h_nat_new

    nc.compile()
    return nc


_NC_CACHE = None


def _get_nc():
    global _NC_CACHE
    if _NC_CACHE is None:
        _NC_CACHE = _build()
    return _NC_CACHE


def _prepare_in_maps(inputs):
    ei = np.asarray(inputs["edge_index"])
    bond = np.asarray(inputs["bond_features"], dtype=np.float32)
    W_emb = np.asarray(inputs["W_emb"], dtype=np.float32)
    b_emb = np.asarray(inputs["b_emb"], dtype=np.float32)
    W_h = np.asarray(inputs["W_h"], dtype=np.float32)
    b_h = np.asarray(inputs["b_h"], dtype=np.float32)
    ln1_g = np.asarray(inputs["ln1_g"], dtype=np.float32)
    ln1_b = np.asarray(inputs["ln1_b"], dtype=np.float32)
    in_w = np.asarray(inputs["in_w"], dtype=np.float32)
    in_b = np.asarray(inputs["in_b"], dtype=np.float32)
    out_w = np.asarray(inputs["out_w"], dtype=np.float32)
    out_b = np.asarray(inputs["out_b"], dtype=np.float32)
    up_w = np.asarray(inputs["up_w"], dtype=np.float32)
    up_b = np.asarray(inputs["up_b"], dtype=np.float32)
    ln2_g = np.asarray(inputs["ln2_g"], dtype=np.float32)
    ln2_b = np.asarray(inputs["ln2_b"], dtype=np.float32)

    tgt = ei[1].astype(np.int64)
    deg = np.zeros(NN, np.float32)
    np.add.at(deg, tgt, 1.0)
    deg_tgt = deg[tgt]  # [E]

    # scale q columns by 1/sqrt(d)
    sc = 1.0 / np.sqrt(np.float32(D))
    in_w_s = in_w.copy()
    in_w_s[:, :, :H] *= sc
    in_b_s = in_b.copy()
    in_b_s[:, :H] *= sc

    shared = {
        "wemb": W_emb.astype(BFNP),
        "bemb": b_emb.reshape(HK, P).T.copy(),
        "wh": W_h.reshape(HK, P, H).transpose(1, 0, 2).astype(BFNP),
        "bh": b_h.reshape(HK, P).T.copy(),
        "inw": in_w_s.reshape(L, HK, P, 3 * H).transpose(2, 0, 1, 3).astype(BFNP),
        "inb": in_b_s.reshape(L, M6, P).transpose(2, 0, 1).copy(),
        "outw": out_w.reshape(L, HK, P, H).transpose(2, 0, 1, 3).astype(BFNP),
        "upw": up_w.reshape(L, HK, P, H).transpose(2, 0, 1, 3).astype(BFNP),
        "upb2": (up_b + np.einsum("lh,lho->lo", out_b, up_w)).astype(np.float32),
        "ln1g": ln1_g, "ln1b": ln1_b, "ln2g": ln2_g, "ln2b": ln2_b,
    }
    shared = {k: np.ascontiguousarray(v) for k, v in shared.items()}

    in_maps = []
    for c in range(NC):
        sl = slice(c * EL, (c + 1) * EL)
        tl = tgt[sl]
        dl = deg_tgt[sl]
        A = np.zeros((EL, NN), np.float32)
        A[np.arange(EL), tl] = 1.0
        B = np.zeros(((NT + EC) * P, EL), np.float32)
        B[tl, np.arange(EL)] = 1.0
        B[NN + np.arange(EL), np.arange(EL)] = -dl
        m = {
            "bondT": np.ascontiguousarray(bond[sl].T.astype(BFNP)),
            "Amat": np.ascontiguousarray(
                A.reshape(EC, P, NN).transpose(1, 0, 2).astype(BFNP)
            ),
            "Bmat": np.ascontiguousarray(
                B.reshape(JB, P, EL).transpose(1, 0, 2).astype(BFNP)
            ),
        }
        m.update(shared)
        in_maps.append(m)
    return in_maps


def kernel(**inputs):
    nc = _get_nc()
    in_maps = _prepare_in_maps(inputs)
    res = run_bass_kernel_spmd(nc, in_maps, core_ids=list(range(NC)))
    out = np.concatenate(
        [np.asarray(res.results[c]["hout"]) for c in range(NC)], axis=0
    )
    return out.astype(np.float32)


# revision 9
# speedup vs baseline: 1.4549x; 1.0470x over previous
"""BondMessagePassing kernel for 8 Trainium2 NeuronCores.

Edge-sharded data parallelism: 512 edges per core. Per layer:
  - node segment-sum via matmul with host-built one-hot + AllReduce
  - gather + residual term via one fused matmul (B = [A^T; -diag(deg)])
  - full-sequence MHA over 4096 edges: each core computes its 512 query
    rows against the AllGathered K/V of all cores
Linears run in transposed-activation layout so weights are natural lhsT;
PE transposes switch layouts where LayerNorm/segment ops need row layout.
"""

import numpy as np
import ml_dtypes

import concourse.bass as bass
import concourse.tile as tile
import concourse.mybir as mybir
from concourse import bacc
from concourse.bass_utils import run_bass_kernel_spmd
from concourse.masks import make_identity

F32 = mybir.dt.float32
BF16 = mybir.dt.bfloat16
AF = mybir.ActivationFunctionType
ALU = mybir.AluOpType
BFNP = ml_dtypes.bfloat16

NC = 8          # cores
P = 128         # partitions
NN = 1024       # nodes
E = 4096        # edges
EL = E // NC    # edges per core (512)
H = 256         # hidden
BD = 64         # bond dim
NH = 8          # heads
D = H // NH     # head dim (32)
L = 3           # layers
HK = H // P     # 2  K-chunks per 256
EC = EL // P    # 4  edge chunks per core
NT = NN // P    # 8  node tiles
KT = E // P     # 32 k-tiles (global edges)
M6 = 3 * H // P  # 6 qkv out tiles
JB = NT + EC    # 12 K-chunks of the fused r matmul
AGW = 1024 + EC * NH * 33  # 2080 allgather row width (K^T 1024 + V_aug 1056)


def _build():
    nc = bacc.Bacc(None, target_bir_lowering=False, num_devices=NC)

    di = {}
    def din(name, shape, dtype):
        di[name] = nc.dram_tensor(name, shape, dtype, kind="ExternalInput")
        return di[name]

    din("bondT", [BD, EL], BF16)
    din("Amat", [P, EC, NN], BF16)
    din("Bmat", [P, JB, EL], BF16)
    din("wemb", [BD, H], BF16)
    din("bemb", [P, HK], F32)
    din("wh", [P, HK, H], BF16)
    din("bh", [P, HK], F32)
    din("inw", [P, L, HK, 3 * H], BF16)
    din("inb", [P, L, M6], F32)
    din("outw", [P, L, HK, H], BF16)
    din("upw", [P, L, HK, H], BF16)
    din("upb2", [L, H], F32)
    din("ln1g", [L, H], F32)
    din("ln1b", [L, H], F32)
    din("ln2g", [L, H], F32)
    din("ln2b", [L, H], F32)
    hout = nc.dram_tensor("hout", [EL, H], F32, kind="ExternalOutput")

    rg = [list(range(NC))]

    with tile.TileContext(nc) as tc:
        with (
            tc.tile_pool(name="const", bufs=1) as const,
            tc.tile_pool(name="sb", bufs=2) as sb,
            tc.tile_pool(name="kv", bufs=1) as kv,
            tc.tile_pool(name="ptp", bufs=4) as ptp,
            tc.tile_pool(name="pmm2", bufs=2, space="PSUM") as pmm2,
            tc.tile_pool(name="pacc", bufs=2, space="PSUM") as pacc,
            tc.tile_pool(name="paux", bufs=2, space="PSUM") as paux,
            tc.tile_pool(name="dram", bufs=1, space="DRAM") as dram,
        ):
            # ---- load constants ----
            bondT_sb = const.tile([BD, EL], BF16)
            nc.sync.dma_start(bondT_sb[:], di["bondT"][:])
            A_sb = const.tile([P, EC, NN], BF16)
            nc.sync.dma_start(A_sb[:], di["Amat"][:])
            B_sb = const.tile([P, JB, EL], BF16)
            nc.sync.dma_start(B_sb[:], di["Bmat"][:])
            wemb_sb = const.tile([BD, H], BF16)
            nc.sync.dma_start(wemb_sb[:], di["wemb"][:])
            bemb_sb = const.tile([P, HK], F32)
            nc.sync.dma_start(bemb_sb[:], di["bemb"][:])
            wh_sb = const.tile([P, HK, H], BF16)
            nc.sync.dma_start(wh_sb[:], di["wh"][:])
            bh_sb = const.tile([P, HK], F32)
            nc.sync.dma_start(bh_sb[:], di["bh"][:])
            inw_sb = const.tile([P, L, HK, 3 * H], BF16)
            nc.sync.dma_start(inw_sb[:], di["inw"][:])
            inb_sb = const.tile([P, L, M6], F32)
            nc.sync.dma_start(inb_sb[:], di["inb"][:])
            outw_sb = const.tile([P, L, HK, H], BF16)
            nc.sync.dma_start(outw_sb[:], di["outw"][:])
            upw_sb = const.tile([P, L, HK, H], BF16)
            nc.sync.dma_start(upw_sb[:], di["upw"][:])

            def bcast_load(name):
                t = const.tile([P, L, H], F32, name=f"{name}_bc")
                src = di[name][:]
                bap = bass.AP(
                    tensor=src.tensor,
                    offset=src.offset,
                    ap=[[0, P]] + [list(x) for x in src.ap],
                )
                nc.sync.dma_start(t[:], bap)
                return t

            upb2_bc = bcast_load("upb2")
            ln1g_bc = bcast_load("ln1g")
            ln1b_bc = bcast_load("ln1b")
            ln2g_bc = bcast_load("ln2g")
            ln2b_bc = bcast_load("ln2b")

            ident_bf = const.tile([P, P], BF16)
            make_identity(nc, ident_bf[:])
            ones_f = const.tile([1, D], F32)
            nc.vector.memset(ones_f[:], 1.0)
            eps_sb = const.tile([P, 1], F32)
            nc.vector.memset(eps_sb[:], 1e-5)

            def transpose_128(dst_ap, src_ap):
                pst = paux.tile([P, P], BF16, tag="aux", name="pst")
                nc.tensor.transpose(pst[:], src_ap, ident_bf[:])
                nc.vector.tensor_copy(dst_ap, pst[:])

            # ---- embedding: h = gelu(bond @ W_emb + b_emb) @ W_h + b_h ----
            g1 = sb.tile([P, HK, EL], BF16, name="g1")
            for m in range(HK):
                ps = paux.tile([P, EL], F32, tag="aux", name="ps_e")
                nc.tensor.matmul(
                    ps[:], wemb_sb[:, m * P:(m + 1) * P], bondT_sb[:],
                    start=True, stop=True,
                )
                nc.scalar.activation(
                    g1[:, m, :], ps[:], AF.Gelu, bias=bemb_sb[:, m:m + 1]
                )
            hT = sb.tile([P, HK, EL], BF16, name="hT")
            for m in range(HK):
                ps = paux.tile([P, EL], F32, tag="aux", name="ps_h")
                for k in range(HK):
                    nc.tensor.matmul(
                        ps[:], wh_sb[:, k, m * P:(m + 1) * P], g1[:, k, :],
                        start=(k == 0), stop=(k == HK - 1),
                    )
                nc.vector.tensor_scalar_add(hT[:, m, :], ps[:], bh_sb[:, m:m + 1])
            h_nat = sb.tile([P, EC, H], BF16, name="h_nat")
            for m in range(HK):
                for c in range(EC):
                    transpose_128(
                        h_nat[:, c, m * P:(m + 1) * P],
                        hT[:, m, c * P:(c + 1) * P],
                    )

            # ---- layers ----
            for t in range(L):
                # A. partial segment-sum over local edges, AllReduce (bf16)
                ar_in = dram.tile([NN, H], BF16, name=f"ar_in{t}")
                ar_out = dram.tile([NN, H], BF16, addr_space="Shared", name=f"ar_out{t}")
                for i in range(NT):
                    ps = paux.tile([P, EL], F32, tag="aux", name="ps_s")
                    for c in range(EC):
                        nc.tensor.matmul(
                            ps[:, :H], A_sb[:, c, i * P:(i + 1) * P], h_nat[:, c, :],
                            start=(c == 0), stop=(c == EC - 1),
                        )
                    s16 = sb.tile([P, H], BF16, tag="s16", name="s16")
                    nc.vector.tensor_copy(s16[:], ps[:, :H])
                    nc.sync.dma_start(ar_in[i * P:(i + 1) * P, :], s16[:])
                nc.gpsimd.collective_compute(
                    "AllReduce", ALU.add, replica_groups=rg,
                    ins=[ar_in[:]], outs=[ar_out[:]],
                )
                s_bf = sb.tile([P, NT, H], BF16, name="s_bf")
                nc.sync.dma_start(
                    s_bf[:],
                    ar_out[:].rearrange("(i p) h -> p i h", p=P),
                )

                # B. r = S[tgt] - deg[tgt]*h  (fused matmul), keep f32
                r_nat = sb.tile([P, EC, H], F32, name="r_nat")
                mv4 = sb.tile([P, EC, 2], F32, name="mv4")
                for m in range(EC):
                    ps = paux.tile([P, EL], F32, tag="aux", name="ps_r")
                    for j in range(JB):
                        rhs = s_bf[:, j, :] if j < NT else h_nat[:, j - NT, :]
                        nc.tensor.matmul(
                            ps[:, :H], B_sb[:, j, m * P:(m + 1) * P], rhs,
                            start=(j == 0), stop=(j == JB - 1),
                        )
                    nc.vector.tensor_copy(r_nat[:, m, :], ps[:, :H])
                    stats = sb.tile([P, 6], F32, tag="stats", name="stats")
                    nc.vector.bn_stats(stats[:], ps[:, :H])
                    nc.vector.bn_aggr(mv4[:, m, :], stats[:])
                # C. LN1 -> xn (bf16): batched rstd then apply
                rstd4 = sb.tile([P, EC], F32, name="rstd4")
                nc.scalar.activation(rstd4[:], mv4[:, :, 1], AF.Sqrt, bias=eps_sb[:])
                nc.vector.reciprocal(rstd4[:], rstd4[:])
                xn_bf = sb.tile([P, EC, H], BF16, name="xn_bf")
                for m in range(EC):
                    xc = sb.tile([P, H], F32, tag="xn32", name="xn32")
                    nc.vector.tensor_scalar(
                        xc[:], r_nat[:, m, :], mv4[:, m, 0:1], rstd4[:, m:m + 1],
                        op0=ALU.subtract, op1=ALU.mult,
                    )
                    nc.vector.tensor_mul(xc[:], xc[:], ln1g_bc[:, t, :])
                    nc.vector.tensor_add(xn_bf[:, m, :], xc[:], ln1b_bc[:, t, :])

                # D. xn^T
                xnT = sb.tile([P, HK, EL], BF16, name="xnT")
                for c in range(EC):
                    for hf in range(HK):
                        transpose_128(
                            xnT[:, hf, c * P:(c + 1) * P],
                            xn_bf[:, c, hf * P:(hf + 1) * P],
                        )

                # E. in-proj: qkv^T = in_w^T @ xn^T + in_b
                QT = sb.tile([P, HK, EL], BF16, name="QT")
                KTl = sb.tile([P, HK, EL], BF16, name="KTl")
                VTl = sb.tile([P, HK, EL], BF16, name="VTl")
                dests = [(QT, 0), (QT, 1), (KTl, 0), (KTl, 1), (VTl, 0), (VTl, 1)]
                for m in range(M6):
                    ps = paux.tile([P, EL], F32, tag="aux", name="ps_q")
                    for k in range(HK):
                        nc.tensor.matmul(
                            ps[:], inw_sb[:, t, k, m * P:(m + 1) * P], xnT[:, k, :],
                            start=(k == 0), stop=(k == HK - 1),
                        )
                    dt_, idx = dests[m]
                    nc.vector.tensor_scalar_add(
                        dt_[:, idx, :], ps[:], inb_sb[:, t, m:m + 1]
                    )

                # F. v natural + ones column (V_aug), local shard
                vnat = sb.tile([P, EC, NH, 33], BF16, name="vnat")
                for hf in range(HK):
                    for c in range(EC):
                        pst = paux.tile([P, P], BF16, tag="aux", name="pst_v")
                        nc.tensor.transpose(
                            pst[:], VTl[:, hf, c * P:(c + 1) * P], ident_bf[:]
                        )
                        nc.vector.tensor_copy(
                            vnat[:, c, hf * 4:(hf + 1) * 4, 0:32],
                            pst[:].rearrange("p (a b) -> p a b", a=4),
                        )
                nc.vector.memset(vnat[:, :, :, 32:33], 1.0)

                # G. AllGather K^T and V_aug
                ag_in = dram.tile([P, AGW], BF16, name=f"ag_in{t}")
                ag_out = dram.tile(
                    [P * NC, AGW], BF16, addr_space="Shared", name=f"ag_out{t}"
                )
                nc.sync.dma_start(
                    ag_in[:, 0:1024].rearrange("p (a b) -> p a b", a=HK), KTl[:]
                )
                nc.sync.dma_start(
                    ag_in[:, 1024:AGW].rearrange(
                        "p (a b c) -> p a b c", a=EC, b=NH
                    ),
                    vnat[:],
                )
                nc.gpsimd.collective_compute(
                    "AllGather", ALU.bypass, replica_groups=rg,
                    ins=[ag_in[:]], outs=[ag_out[:]],
                )
                KT_all = kv.tile([P, NC, HK, EL], BF16, name="KT_all")
                V_all = kv.tile([P, NC, EC, NH, 33], BF16, name="V_all")
                for s in range(NC):
                    nc.sync.dma_start(
                        KT_all[:, s, :, :],
                        ag_out[s * P:(s + 1) * P, 0:1024].rearrange(
                            "p (a b) -> p a b", a=HK
                        ),
                    )
                    nc.sync.dma_start(
                        V_all[:, s, :, :, :],
                        ag_out[s * P:(s + 1) * P, 1024:AGW].rearrange(
                            "p (a b c) -> p a b c", a=EC, b=NH
                        ),
                    )

                # H. attention: pairs of heads on disjoint PE quadrants so the
                #    K=32 QK matmuls overlap in the array; scores for the pair
                #    share one 2-bank PSUM tile -> single batched exp; PV with
                #    ones row gives the softmax denominator.
                oT = sb.tile([P, HK, EL], BF16, name="oT")
                for hA, hB in ((0, 2), (1, 3), (4, 6), (5, 7)):
                    pair = (hA, hB)
                    # two accumulators in separate banks at disjoint PE array
                    # columns (0 / 64) so the pair's PV matmuls overlap
                    accs = [
                        pacc.tile([P, EL], F32, tag="acc", name="ps_o")
                        for _ in range(2)
                    ]
                    for kt in range(KT):
                        s, c = divmod(kt, EC)
                        ps2 = pmm2.tile([P, 2, EL], F32, tag="mm", name="ps2")
                        for j, h in enumerate(pair):
                            hp = (h % 4) * D
                            hf = h // 4
                            nc.tensor.matmul(
                                ps2[:, j, :],
                                KT_all[hp:hp + D, s, hf, c * P:(c + 1) * P],
                                QT[hp:hp + D, hf, :],
                                start=True, stop=True,
                                tile_position=(hp, 0),
                            )
                        pt2 = ptp.tile([P, 2, EL], BF16, tag="pt", name="pt")
                        nc.scalar.activation(pt2[:], ps2[:], AF.Exp)
                        for j, h in enumerate(pair):
                            cb = j * 64
                            nc.tensor.matmul(
                                accs[j][cb:cb + 33, :],
                                V_all[:, s, c, h, 0:33], pt2[:, j, :],
                                start=(kt == 0), stop=(kt == KT - 1),
                                tile_position=(0, cb),
                            )
                    for j, h in enumerate(pair):
                        hp = (h % 4) * D
                        hf = h // 4
                        cb = j * 64
                        dnr = sb.tile([1, EL], F32, tag="dnr", name="dnr")
                        nc.vector.tensor_copy(dnr[:], accs[j][cb + 32:cb + 33, :])
                        den = sb.tile([1, EL], F32, tag="den", name="den")
                        nc.vector.reciprocal_approx_fast(den[:], dnr[:])
                        ps_b = paux.tile([D, EL], F32, tag="aux", name="ps_b")
                        nc.tensor.matmul(
                            ps_b[:], ones_f[:], den[:], start=True, stop=True
                        )
                        rec32 = sb.tile([D, EL], F32, tag="rec32", name="rec32")
                        nc.vector.tensor_copy(rec32[:], ps_b[:])
                        nc.vector.tensor_mul(
                            oT[hp:hp + D, hf, :], accs[j][cb:cb + 32, :], rec32[:]
                        )

                # I. out-proj + residual: t_ij = attn + 2r (out_b folded into up_b)
                t_bf = sb.tile([P, EC, H], BF16, name="t_bf")
                for m in range(EC):
                    ps = paux.tile([P, EL], F32, tag="aux", name="ps_a")
                    for k in range(HK):
                        nc.tensor.matmul(
                            ps[:, :H], oT[:, k, m * P:(m + 1) * P], outw_sb[:, t, k, :],
                            start=(k == 0), stop=(k == HK - 1),
                        )
                    nc.vector.scalar_tensor_tensor(
                        t_bf[:, m, :], r_nat[:, m, :], 2.0, ps[:, :H],
                        op0=ALU.mult, op1=ALU.add,
                    )

                # J. t^T
                tT = sb.tile([P, HK, EL], BF16, name="tT")
                for c in range(EC):
                    for hf in range(HK):
                        transpose_128(
                            tT[:, hf, c * P:(c + 1) * P],
                            t_bf[:, c, hf * P:(hf + 1) * P],
                        )

                # K. up-proj + LN2 + gelu -> next h (or output)
                last = t == L - 1
                if not last:
                    h_nat_new = sb.tile([P, EC, H], BF16, name="h_nat")
                u4 = sb.tile([P, EC, H], F32, name="u4")
                mv4b = sb.tile([P, EC, 2], F32, name="mv4b")
                for m in range(EC):
                    ps = paux.tile([P, EL], F32, tag="aux", name="ps_u")
                    for k in range(HK):
                        nc.tensor.matmul(
                            ps[:, :H], tT[:, k, m * P:(m + 1) * P], upw_sb[:, t, k, :],
                            start=(k == 0), stop=(k == HK - 1),
                        )
                    nc.vector.tensor_add(u4[:, m, :], ps[:, :H], upb2_bc[:, t, :])
                    stats = sb.tile([P, 6], F32, tag="stats", name="stats")
                    nc.vector.bn_stats(stats[:], u4[:, m, :])
                    nc.vector.bn_aggr(mv4b[:, m, :], stats[:])
                rstd4b = sb.tile([P, EC], F32, name="rstd4b")
                nc.scalar.activation(rstd4b[:], mv4b[:, :, 1], AF.Sqrt, bias=eps_sb[:])
                nc.vector.reciprocal(rstd4b[:], rstd4b[:])
                for m in range(EC):
                    xc = sb.tile([P, H], F32, tag="xln", name="xln")
                    nc.vector.tensor_scalar(
                        xc[:], u4[:, m, :], mv4b[:, m, 0:1], rstd4b[:, m:m + 1],
                        op0=ALU.subtract, op1=ALU.mult,
                    )
                    nc.vector.tensor_mul(xc[:], xc[:], ln2g_bc[:, t, :])
                    uln = sb.tile([P, H], F32, tag="uln", name="uln")
                    nc.vector.tensor_add(uln[:], xc[:], ln2b_bc[:, t, :])
                    if last:
                        hf32 = sb.tile([P, H], F32, tag="hf32", name="hf32")
                        nc.scalar.activation(hf32[:], uln[:], AF.Gelu)
                        nc.sync.dma_start(hout[m * P:(m + 1) * P, :], hf32[:])
                    else:
                        nc.scalar.activation(h_nat_new[:, m, :], uln[:], AF.Gelu)
                if not last:
                    h_nat = h_nat_new

    nc.compile()
    return nc


_NC_CACHE = None


def _get_nc():
    global _NC_CACHE
    if _NC_CACHE is None:
        _NC_CACHE = _build()
    return _NC_CACHE


def _prepare_in_maps(inputs):
    ei = np.asarray(inputs["edge_index"])
    bond = np.asarray(inputs["bond_features"], dtype=np.float32)
    W_emb = np.asarray(inputs["W_emb"], dtype=np.float32)
    b_emb = np.asarray(inputs["b_emb"], dtype=np.float32)
    W_h = np.asarray(inputs["W_h"], dtype=np.float32)
    b_h = np.asarray(inputs["b_h"], dtype=np.float32)
    ln1_g = np.asarray(inputs["ln1_g"], dtype=np.float32)
    ln1_b = np.asarray(inputs["ln1_b"], dtype=np.float32)
    in_w = np.asarray(inputs["in_w"], dtype=np.float32)
    in_b = np.asarray(inputs["in_b"], dtype=np.float32)
    out_w = np.asarray(inputs["out_w"], dtype=np.float32)
    out_b = np.asarray(inputs["out_b"], dtype=np.float32)
    up_w = np.asarray(inputs["up_w"], dtype=np.float32)
    up_b = np.asarray(inputs["up_b"], dtype=np.float32)
    ln2_g = np.asarray(inputs["ln2_g"], dtype=np.float32)
    ln2_b = np.asarray(inputs["ln2_b"], dtype=np.float32)

    tgt = ei[1].astype(np.int64)
    deg = np.zeros(NN, np.float32)
    np.add.at(deg, tgt, 1.0)
    deg_tgt = deg[tgt]  # [E]

    # scale q columns by 1/sqrt(d)
    sc = 1.0 / np.sqrt(np.float32(D))
    in_w_s = in_w.copy()
    in_w_s[:, :, :H] *= sc
    in_b_s = in_b.copy()
    in_b_s[:, :H] *= sc

    shared = {
        "wemb": W_emb.astype(BFNP),
        "bemb": b_emb.reshape(HK, P).T.copy(),
        "wh": W_h.reshape(HK, P, H).transpose(1, 0, 2).astype(BFNP),
        "bh": b_h.reshape(HK, P).T.copy(),
        "inw": in_w_s.reshape(L, HK, P, 3 * H).transpose(2, 0, 1, 3).astype(BFNP),
        "inb": in_b_s.reshape(L, M6, P).transpose(2, 0, 1).copy(),
        "outw": out_w.reshape(L, HK, P, H).transpose(2, 0, 1, 3).astype(BFNP),
        "upw": up_w.reshape(L, HK, P, H).transpose(2, 0, 1, 3).astype(BFNP),
        "upb2": (up_b + np.einsum("lh,lho->lo", out_b, up_w)).astype(np.float32),
        "ln1g": ln1_g, "ln1b": ln1_b, "ln2g": ln2_g, "ln2b": ln2_b,
    }
    shared = {k: np.ascontiguousarray(v) for k, v in shared.items()}

    in_maps = []
    for c in range(NC):
        sl = slice(c * EL, (c + 1) * EL)
        tl = tgt[sl]
        dl = deg_tgt[sl]
        A = np.zeros((EL, NN), np.float32)
        A[np.arange(EL), tl] = 1.0
        B = np.zeros(((NT + EC) * P, EL), np.float32)
        B[tl, np.arange(EL)] = 1.0
        B[NN + np.arange(EL), np.arange(EL)] = -dl
        m = {
            "bondT": np.ascontiguousarray(bond[sl].T.astype(BFNP)),
            "Amat": np.ascontiguousarray(
                A.reshape(EC, P, NN).transpose(1, 0, 2).astype(BFNP)
            ),
            "Bmat": np.ascontiguousarray(
                B.reshape(JB, P, EL).transpose(1, 0, 2).astype(BFNP)
            ),
        }
        m.update(shared)
        in_maps.append(m)
    return in_maps


def kernel(**inputs):
    nc = _get_nc()
    in_maps = _prepare_in_maps(inputs)
    res = run_bass_kernel_spmd(nc, in_maps, core_ids=list(range(NC)))
    out = np.concatenate(
        [np.asarray(res.results[c]["hout"]) for c in range(NC)], axis=0
    )
    return out.astype(np.float32)


# revision 12
# speedup vs baseline: 1.5145x; 1.0410x over previous
"""BondMessagePassing kernel for 8 Trainium2 NeuronCores.

Edge-sharded data parallelism: 512 edges per core. Per layer:
  - node segment-sum via matmul with host-built one-hot + AllReduce
  - gather + residual term via one fused matmul (B = [A^T; -diag(deg)])
  - full-sequence MHA over 4096 edges: each core computes its 512 query
    rows against the AllGathered K/V of all cores
Linears run in transposed-activation layout so weights are natural lhsT;
PE transposes switch layouts where LayerNorm/segment ops need row layout.
"""

import numpy as np
import ml_dtypes

import concourse.bass as bass
import concourse.tile as tile
import concourse.mybir as mybir
from concourse import bacc
from concourse.bass_utils import run_bass_kernel_spmd
from concourse.masks import make_identity

F32 = mybir.dt.float32
BF16 = mybir.dt.bfloat16
AF = mybir.ActivationFunctionType
ALU = mybir.AluOpType
BFNP = ml_dtypes.bfloat16

NC = 8          # cores
P = 128         # partitions
NN = 1024       # nodes
E = 4096        # edges
EL = E // NC    # edges per core (512)
H = 256         # hidden
BD = 64         # bond dim
NH = 8          # heads
D = H // NH     # head dim (32)
L = 3           # layers
HK = H // P     # 2  K-chunks per 256
EC = EL // P    # 4  edge chunks per core
NT = NN // P    # 8  node tiles
KT = E // P     # 32 k-tiles (global edges)
M6 = 3 * H // P  # 6 qkv out tiles
JB = NT + EC    # 12 K-chunks of the fused r matmul
AGW = 1024 + EC * NH * 33  # 2080 allgather row width (K^T 1024 + V_aug 1056)


def _build():
    nc = bacc.Bacc(None, target_bir_lowering=False, num_devices=NC)

    di = {}
    def din(name, shape, dtype):
        di[name] = nc.dram_tensor(name, shape, dtype, kind="ExternalInput")
        return di[name]

    din("bondT", [BD, EL], BF16)
    din("Amat", [P, EC, NN], BF16)
    din("Bmat", [P, JB, EL], BF16)
    din("wemb", [BD, H], BF16)
    din("bemb", [P, HK], F32)
    din("wh", [P, HK, H], BF16)
    din("bh", [P, HK], F32)
    din("inw", [P, L, HK, 3 * H], BF16)
    din("inb", [P, L, M6], F32)
    din("outw", [P, L, HK, H], BF16)
    din("upw", [P, L, HK, H], BF16)
    din("upb2", [L, H], F32)
    din("ln1g", [L, H], F32)
    din("ln1b", [L, H], F32)
    din("ln2g", [L, H], F32)
    din("ln2b", [L, H], F32)
    hout = nc.dram_tensor("hout", [EL, H], F32, kind="ExternalOutput")

    rg = [list(range(NC))]

    with tile.TileContext(nc) as tc:
        with (
            tc.tile_pool(name="const", bufs=1) as const,
            tc.tile_pool(name="sb", bufs=2) as sb,
            tc.tile_pool(name="kv", bufs=1) as kv,
            tc.tile_pool(name="ptp", bufs=4) as ptp,
            tc.tile_pool(name="pmm2", bufs=2, space="PSUM") as pmm2,
            tc.tile_pool(name="pacc", bufs=2, space="PSUM") as pacc,
            tc.tile_pool(name="paux", bufs=2, space="PSUM") as paux,
            tc.tile_pool(name="dram", bufs=1, space="DRAM") as dram,
        ):
            # ---- load constants ----
            bondT_sb = const.tile([BD, EL], BF16)
            nc.sync.dma_start(bondT_sb[:], di["bondT"][:])
            A_sb = const.tile([P, EC, NN], BF16)
            nc.sync.dma_start(A_sb[:], di["Amat"][:])
            B_sb = const.tile([P, JB, EL], BF16)
            nc.sync.dma_start(B_sb[:], di["Bmat"][:])
            wemb_sb = const.tile([BD, H], BF16)
            nc.sync.dma_start(wemb_sb[:], di["wemb"][:])
            bemb_sb = const.tile([P, HK], F32)
            nc.sync.dma_start(bemb_sb[:], di["bemb"][:])
            wh_sb = const.tile([P, HK, H], BF16)
            nc.sync.dma_start(wh_sb[:], di["wh"][:])
            bh_sb = const.tile([P, HK], F32)
            nc.sync.dma_start(bh_sb[:], di["bh"][:])
            inw_sb = const.tile([P, L, HK, 3 * H], BF16)
            nc.sync.dma_start(inw_sb[:], di["inw"][:])
            inb_sb = const.tile([P, L, M6], F32)
            nc.sync.dma_start(inb_sb[:], di["inb"][:])
            outw_sb = const.tile([P, L, HK, H], BF16)
            nc.sync.dma_start(outw_sb[:], di["outw"][:])
            upw_sb = const.tile([P, L, HK, H], BF16)
            nc.sync.dma_start(upw_sb[:], di["upw"][:])

            def bcast_load(name):
                t = const.tile([P, L, H], F32, name=f"{name}_bc")
                src = di[name][:]
                bap = bass.AP(
                    tensor=src.tensor,
                    offset=src.offset,
                    ap=[[0, P]] + [list(x) for x in src.ap],
                )
                nc.sync.dma_start(t[:], bap)
                return t

            upb2_bc = bcast_load("upb2")
            ln1g_bc = bcast_load("ln1g")
            ln1b_bc = bcast_load("ln1b")
            ln2g_bc = bcast_load("ln2g")
            ln2b_bc = bcast_load("ln2b")

            ident_bf = const.tile([P, P], BF16)
            make_identity(nc, ident_bf[:])
            ones_f = const.tile([1, D], F32)
            nc.vector.memset(ones_f[:], 1.0)
            eps_sb = const.tile([P, 1], F32)
            nc.vector.memset(eps_sb[:], 1e-5)

            def transpose_128(dst_ap, src_ap):
                pst = paux.tile([P, P], BF16, tag="aux", name="pst")
                nc.tensor.transpose(pst[:], src_ap, ident_bf[:])
                nc.vector.tensor_copy(dst_ap, pst[:])

            # ---- embedding: h = gelu(bond @ W_emb + b_emb) @ W_h + b_h ----
            g1 = sb.tile([P, HK, EL], BF16, name="g1")
            for m in range(HK):
                ps = paux.tile([P, EL], F32, tag="aux", name="ps_e")
                nc.tensor.matmul(
                    ps[:], wemb_sb[:, m * P:(m + 1) * P], bondT_sb[:],
                    start=True, stop=True,
                )
                nc.scalar.activation(
                    g1[:, m, :], ps[:], AF.Gelu, bias=bemb_sb[:, m:m + 1]
                )
            hT = sb.tile([P, HK, EL], BF16, name="hT")
            for m in range(HK):
                ps = paux.tile([P, EL], F32, tag="aux", name="ps_h")
                for k in range(HK):
                    nc.tensor.matmul(
                        ps[:], wh_sb[:, k, m * P:(m + 1) * P], g1[:, k, :],
                        start=(k == 0), stop=(k == HK - 1),
                    )
                nc.vector.tensor_scalar_add(hT[:, m, :], ps[:], bh_sb[:, m:m + 1])
            h_nat = sb.tile([P, EC, H], BF16, name="h_nat")
            for m in range(HK):
                for c in range(EC):
                    transpose_128(
                        h_nat[:, c, m * P:(m + 1) * P],
                        hT[:, m, c * P:(c + 1) * P],
                    )

            # ---- layers ----
            for t in range(L):
                # A. partial segment-sum over local edges, AllReduce (bf16)
                ar_in = dram.tile([NN, H], BF16, name=f"ar_in{t}")
                ar_out = dram.tile([NN, H], BF16, addr_space="Shared", name=f"ar_out{t}")
                for i in range(NT):
                    ps = paux.tile([P, EL], F32, tag="aux", name="ps_s")
                    for c in range(EC):
                        nc.tensor.matmul(
                            ps[:, :H], A_sb[:, c, i * P:(i + 1) * P], h_nat[:, c, :],
                            start=(c == 0), stop=(c == EC - 1),
                        )
                    s16 = sb.tile([P, H], BF16, tag="s16", name="s16")
                    nc.vector.tensor_copy(s16[:], ps[:, :H])
                    nc.sync.dma_start(ar_in[i * P:(i + 1) * P, :], s16[:])
                nc.gpsimd.collective_compute(
                    "AllReduce", ALU.add, replica_groups=rg,
                    ins=[ar_in[:]], outs=[ar_out[:]],
                )
                s_bf = sb.tile([P, NT, H], BF16, name="s_bf")
                nc.sync.dma_start(
                    s_bf[:],
                    ar_out[:].rearrange("(i p) h -> p i h", p=P),
                )

                # B. r = S[tgt] - deg[tgt]*h  (fused matmul), keep f32
                r_nat = sb.tile([P, EC, H], F32, name="r_nat")
                mv4 = sb.tile([P, EC, 2], F32, name="mv4")
                for m in range(EC):
                    ps = paux.tile([P, EL], F32, tag="aux", name="ps_r")
                    for j in range(JB):
                        rhs = s_bf[:, j, :] if j < NT else h_nat[:, j - NT, :]
                        nc.tensor.matmul(
                            ps[:, :H], B_sb[:, j, m * P:(m + 1) * P], rhs,
                            start=(j == 0), stop=(j == JB - 1),
                        )
                    nc.vector.tensor_copy(r_nat[:, m, :], ps[:, :H])
                    stats = sb.tile([P, 6], F32, tag="stats", name="stats")
                    nc.vector.bn_stats(stats[:], ps[:, :H])
                    nc.vector.bn_aggr(mv4[:, m, :], stats[:])
                # C. LN1 -> xn (bf16): batched rstd then apply
                rstd4 = sb.tile([P, EC], F32, name="rstd4")
                nc.scalar.activation(rstd4[:], mv4[:, :, 1], AF.Sqrt, bias=eps_sb[:])
                nc.vector.reciprocal(rstd4[:], rstd4[:])
                xn_bf = sb.tile([P, EC, H], BF16, name="xn_bf")
                for m in range(EC):
                    xc = sb.tile([P, H], F32, tag="xn32", name="xn32")
                    nc.vector.tensor_scalar(
                        xc[:], r_nat[:, m, :], mv4[:, m, 0:1], rstd4[:, m:m + 1],
                        op0=ALU.subtract, op1=ALU.mult,
                    )
                    nc.vector.tensor_mul(xc[:], xc[:], ln1g_bc[:, t, :])
                    nc.vector.tensor_add(xn_bf[:, m, :], xc[:], ln1b_bc[:, t, :])

                # D. xn^T
                xnT = sb.tile([P, HK, EL], BF16, name="xnT")
                for c in range(EC):
                    for hf in range(HK):
                        transpose_128(
                            xnT[:, hf, c * P:(c + 1) * P],
                            xn_bf[:, c, hf * P:(hf + 1) * P],
                        )

                # E. in-proj: qkv^T = in_w^T @ xn^T + in_b
                QT = sb.tile([P, HK, EL], BF16, name="QT")
                KTl = sb.tile([P, HK, EL], BF16, name="KTl")
                VTl = sb.tile([P, HK, EL], BF16, name="VTl")
                dests = [(QT, 0), (QT, 1), (KTl, 0), (KTl, 1), (VTl, 0), (VTl, 1)]
                for m in range(M6):
                    ps = paux.tile([P, EL], F32, tag="aux", name="ps_q")
                    for k in range(HK):
                        nc.tensor.matmul(
                            ps[:], inw_sb[:, t, k, m * P:(m + 1) * P], xnT[:, k, :],
                            start=(k == 0), stop=(k == HK - 1),
                        )
                    dt_, idx = dests[m]
                    nc.vector.tensor_scalar_add(
                        dt_[:, idx, :], ps[:], inb_sb[:, t, m:m + 1]
                    )

                # F. v natural + ones column (V_aug), local shard
                vnat = sb.tile([P, EC, NH, 33], BF16, name="vnat")
                for hf in range(HK):
                    for c in range(EC):
                        pst = paux.tile([P, P], BF16, tag="aux", name="pst_v")
                        nc.tensor.transpose(
                            pst[:], VTl[:, hf, c * P:(c + 1) * P], ident_bf[:]
                        )
                        nc.vector.tensor_copy(
                            vnat[:, c, hf * 4:(hf + 1) * 4, 0:32],
                            pst[:].rearrange("p (a b) -> p a b", a=4),
                        )
                nc.vector.memset(vnat[:, :, :, 32:33], 1.0)

                # G. AllGather K^T and V_aug
                ag_in = dram.tile([P, AGW], BF16, name=f"ag_in{t}")
                ag_out = dram.tile(
                    [P * NC, AGW], BF16, addr_space="Shared", name=f"ag_out{t}"
                )
                nc.sync.dma_start(
                    ag_in[:, 0:1024].rearrange("p (a b) -> p a b", a=HK), KTl[:]
                )
                nc.sync.dma_start(
                    ag_in[:, 1024:AGW].rearrange(
                        "p (a b c) -> p a b c", a=EC, b=NH
                    ),
                    vnat[:],
                )
                nc.gpsimd.collective_compute(
                    "AllGather", ALU.bypass, replica_groups=rg,
                    ins=[ag_in[:]], outs=[ag_out[:]],
                )
                KT_s = []
                V_s = []
                for s in range(NC):
                    kts = kv.tile([P, HK, EL], BF16, name=f"kt{s}", tag=f"kt{s}")
                    nc.sync.dma_start(
                        kts[:],
                        ag_out[s * P:(s + 1) * P, 0:1024].rearrange(
                            "p (a b) -> p a b", a=HK
                        ),
                    )
                    vs = kv.tile([P, EC, NH, 33], BF16, name=f"v{s}", tag=f"v{s}")
                    nc.sync.dma_start(
                        vs[:],
                        ag_out[s * P:(s + 1) * P, 1024:AGW].rearrange(
                            "p (a b c) -> p a b c", a=EC, b=NH
                        ),
                    )
                    KT_s.append(kts)
                    V_s.append(vs)

                # H. attention: pairs of heads on disjoint PE quadrants so the
                #    K=32 QK matmuls overlap in the array; scores for the pair
                #    share one 2-bank PSUM tile -> single batched exp; PV with
                #    ones row gives the softmax denominator.
                oT = sb.tile([P, HK, EL], BF16, name="oT")
                for hA, hB in ((0, 2), (1, 3), (4, 6), (5, 7)):
                    pair = (hA, hB)
                    # two accumulators in separate banks at disjoint PE array
                    # columns (0 / 64) so the pair's PV matmuls overlap
                    accs = [
                        pacc.tile([P, EL], F32, tag="acc", name="ps_o")
                        for _ in range(2)
                    ]
                    for kt in range(KT):
                        s, c = divmod(kt, EC)
                        ps2 = pmm2.tile([P, 2, EL], F32, tag="mm", name="ps2")
                        for j, h in enumerate(pair):
                            hp = (h % 4) * D
                            hf = h // 4
                            nc.tensor.matmul(
                                ps2[:, j, :],
                                KT_s[s][hp:hp + D, hf, c * P:(c + 1) * P],
                                QT[hp:hp + D, hf, :],
                                start=True, stop=True,
                                tile_position=(hp, 0),
                            )
                        pt2 = ptp.tile([P, 2, EL], BF16, tag="pt", name="pt")
                        nc.scalar.activation(pt2[:], ps2[:], AF.Exp)
                        for j, h in enumerate(pair):
                            cb = j * 64
                            nc.tensor.matmul(
                                accs[j][cb:cb + 33, :],
                                V_s[s][:, c, h, 0:33], pt2[:, j, :],
                                start=(kt == 0), stop=(kt == KT - 1),
                                tile_position=(0, cb),
                            )
                    for j, h in enumerate(pair):
                        hp = (h % 4) * D
                        hf = h // 4
                        cb = j * 64
                        dnr = sb.tile([1, EL], F32, tag="dnr", name="dnr")
                        nc.vector.tensor_copy(dnr[:], accs[j][cb + 32:cb + 33, :])
                        den = sb.tile([1, EL], F32, tag="den", name="den")
                        nc.vector.reciprocal_approx_fast(den[:], dnr[:])
                        rec32 = sb.tile([D, EL], F32, tag="rec32", name="rec32")
                        nc.gpsimd.partition_broadcast(rec32[:], den[:])
                        nc.vector.tensor_mul(
                            oT[hp:hp + D, hf, :], accs[j][cb:cb + 32, :], rec32[:]
                        )

                # I. out-proj + residual: t_ij = attn + 2r (out_b folded into up_b)
                t_bf = sb.tile([P, EC, H], BF16, name="t_bf")
                for m in range(EC):
                    ps = paux.tile([P, EL], F32, tag="aux", name="ps_a")
                    for k in range(HK):
                        nc.tensor.matmul(
                            ps[:, :H], oT[:, k, m * P:(m + 1) * P], outw_sb[:, t, k, :],
                            start=(k == 0), stop=(k == HK - 1),
                        )
                    nc.vector.scalar_tensor_tensor(
                        t_bf[:, m, :], r_nat[:, m, :], 2.0, ps[:, :H],
                        op0=ALU.mult, op1=ALU.add,
                    )

                # J. t^T
                tT = sb.tile([P, HK, EL], BF16, name="tT")
                for c in range(EC):
                    for hf in range(HK):
                        transpose_128(
                            tT[:, hf, c * P:(c + 1) * P],
                            t_bf[:, c, hf * P:(hf + 1) * P],
                        )

                # K. up-proj + LN2 + gelu -> next h (or output)
                last = t == L - 1
                if not last:
                    h_nat_new = sb.tile([P, EC, H], BF16, name="h_nat")
                u4 = sb.tile([P, EC, H], F32, name="u4")
                mv4b = sb.tile([P, EC, 2], F32, name="mv4b")
                for m in range(EC):
                    ps = paux.tile([P, EL], F32, tag="aux", name="ps_u")
                    for k in range(HK):
                        nc.tensor.matmul(
                            ps[:, :H], tT[:, k, m * P:(m + 1) * P], upw_sb[:, t, k, :],
                            start=(k == 0), stop=(k == HK - 1),
                        )
                    nc.vector.tensor_add(u4[:, m, :], ps[:, :H], upb2_bc[:, t, :])
                    stats = sb.tile([P, 6], F32, tag="stats", name="stats")
                    nc.vector.bn_stats(stats[:], u4[:, m, :])
                    nc.vector.bn_aggr(mv4b[:, m, :], stats[:])
                rstd4b = sb.tile([P, EC], F32, name="rstd4b")
                nc.scalar.activation(rstd4b[:], mv4b[:, :, 1], AF.Sqrt, bias=eps_sb[:])
                nc.vector.reciprocal(rstd4b[:], rstd4b[:])
                for m in range(EC):
                    xc = sb.tile([P, H], F32, tag="xln", name="xln")
                    nc.vector.tensor_scalar(
                        xc[:], u4[:, m, :], mv4b[:, m, 0:1], rstd4b[:, m:m + 1],
                        op0=ALU.subtract, op1=ALU.mult,
                    )
                    nc.vector.tensor_mul(xc[:], xc[:], ln2g_bc[:, t, :])
                    uln = sb.tile([P, H], F32, tag="uln", name="uln")
                    nc.vector.tensor_add(uln[:], xc[:], ln2b_bc[:, t, :])
                    if last:
                        hf32 = sb.tile([P, H], F32, tag="hf32", name="hf32")
                        nc.scalar.activation(hf32[:], uln[:], AF.Gelu)
                        nc.sync.dma_start(hout[m * P:(m + 1) * P, :], hf32[:])
                    else:
                        nc.scalar.activation(h_nat_new[:, m, :], uln[:], AF.Gelu)
                if not last:
                    h_nat = h_nat_new

    nc.compile()
    return nc


_NC_CACHE = None


def _get_nc():
    global _NC_CACHE
    if _NC_CACHE is None:
        _NC_CACHE = _build()
    return _NC_CACHE


def _prepare_in_maps(inputs):
    ei = np.asarray(inputs["edge_index"])
    bond = np.asarray(inputs["bond_features"], dtype=np.float32)
    W_emb = np.asarray(inputs["W_emb"], dtype=np.float32)
    b_emb = np.asarray(inputs["b_emb"], dtype=np.float32)
    W_h = np.asarray(inputs["W_h"], dtype=np.float32)
    b_h = np.asarray(inputs["b_h"], dtype=np.float32)
    ln1_g = np.asarray(inputs["ln1_g"], dtype=np.float32)
    ln1_b = np.asarray(inputs["ln1_b"], dtype=np.float32)
    in_w = np.asarray(inputs["in_w"], dtype=np.float32)
    in_b = np.asarray(inputs["in_b"], dtype=np.float32)
    out_w = np.asarray(inputs["out_w"], dtype=np.float32)
    out_b = np.asarray(inputs["out_b"], dtype=np.float32)
    up_w = np.asarray(inputs["up_w"], dtype=np.float32)
    up_b = np.asarray(inputs["up_b"], dtype=np.float32)
    ln2_g = np.asarray(inputs["ln2_g"], dtype=np.float32)
    ln2_b = np.asarray(inputs["ln2_b"], dtype=np.float32)

    tgt = ei[1].astype(np.int64)
    deg = np.zeros(NN, np.float32)
    np.add.at(deg, tgt, 1.0)
    deg_tgt = deg[tgt]  # [E]

    # scale q columns by 1/sqrt(d)
    sc = 1.0 / np.sqrt(np.float32(D))
    in_w_s = in_w.copy()
    in_w_s[:, :, :H] *= sc
    in_b_s = in_b.copy()
    in_b_s[:, :H] *= sc

    shared = {
        "wemb": W_emb.astype(BFNP),
        "bemb": b_emb.reshape(HK, P).T.copy(),
        "wh": W_h.reshape(HK, P, H).transpose(1, 0, 2).astype(BFNP),
        "bh": b_h.reshape(HK, P).T.copy(),
        "inw": in_w_s.reshape(L, HK, P, 3 * H).transpose(2, 0, 1, 3).astype(BFNP),
        "inb": in_b_s.reshape(L, M6, P).transpose(2, 0, 1).copy(),
        "outw": out_w.reshape(L, HK, P, H).transpose(2, 0, 1, 3).astype(BFNP),
        "upw": up_w.reshape(L, HK, P, H).transpose(2, 0, 1, 3).astype(BFNP),
        "upb2": (up_b + np.einsum("lh,lho->lo", out_b, up_w)).astype(np.float32),
        "ln1g": ln1_g, "ln1b": ln1_b, "ln2g": ln2_g, "ln2b": ln2_b,
    }
    shared = {k: np.ascontiguousarray(v) for k, v in shared.items()}

    in_maps = []
    for c in range(NC):
        sl = slice(c * EL, (c + 1) * EL)
        tl = tgt[sl]
        dl = deg_tgt[sl]
        A = np.zeros((EL, NN), np.float32)
        A[np.arange(EL), tl] = 1.0
        B = np.zeros(((NT + EC) * P, EL), np.float32)
        B[tl, np.arange(EL)] = 1.0
        B[NN + np.arange(EL), np.arange(EL)] = -dl
        m = {
            "bondT": np.ascontiguousarray(bond[sl].T.astype(BFNP)),
            "Amat": np.ascontiguousarray(
                A.reshape(EC, P, NN).transpose(1, 0, 2).astype(BFNP)
            ),
            "Bmat": np.ascontiguousarray(
                B.reshape(JB, P, EL).transpose(1, 0, 2).astype(BFNP)
            ),
        }
        m.update(shared)
        in_maps.append(m)
    return in_maps


def kernel(**inputs):
    nc = _get_nc()
    in_maps = _prepare_in_maps(inputs)
    res = run_bass_kernel_spmd(nc, in_maps, core_ids=list(range(NC)))
    out = np.concatenate(
        [np.asarray(res.results[c]["hout"]) for c in range(NC)], axis=0
    )
    return out.astype(np.float32)


# revision 14
# speedup vs baseline: 1.5977x; 1.0549x over previous
"""BondMessagePassing kernel for 8 Trainium2 NeuronCores.

Edge-sharded data parallelism: 512 edges per core. Per layer:
  - node segment-sum via matmul with host-built one-hot + AllReduce
  - gather + residual term via one fused matmul (B = [A^T; -diag(deg)])
  - full-sequence MHA over 4096 edges: each core computes its 512 query
    rows against the AllGathered K/V of all cores
Linears run in transposed-activation layout so weights are natural lhsT;
PE transposes switch layouts where LayerNorm/segment ops need row layout.
"""

import numpy as np
import ml_dtypes

import concourse.bass as bass
import concourse.tile as tile
import concourse.mybir as mybir
from concourse import bacc
from concourse.bass_utils import run_bass_kernel_spmd
from concourse.masks import make_identity

F32 = mybir.dt.float32
BF16 = mybir.dt.bfloat16
AF = mybir.ActivationFunctionType
ALU = mybir.AluOpType
BFNP = ml_dtypes.bfloat16

NC = 8          # cores
P = 128         # partitions
NN = 1024       # nodes
E = 4096        # edges
EL = E // NC    # edges per core (512)
H = 256         # hidden
BD = 64         # bond dim
NH = 8          # heads
D = H // NH     # head dim (32)
L = 3           # layers
HK = H // P     # 2  K-chunks per 256
EC = EL // P    # 4  edge chunks per core
NT = NN // P    # 8  node tiles
KT = E // P     # 32 k-tiles (global edges)
M6 = 3 * H // P  # 6 qkv out tiles
JB = NT + EC    # 12 K-chunks of the fused r matmul
AGW = 1024 + EC * NH * 33  # 2080 allgather row width (K^T 1024 + V_aug 1056)


def _build():
    nc = bacc.Bacc(None, target_bir_lowering=False, num_devices=NC)

    di = {}
    def din(name, shape, dtype):
        di[name] = nc.dram_tensor(name, shape, dtype, kind="ExternalInput")
        return di[name]

    din("bondT", [BD, EL], BF16)
    din("Amat", [P, EC, NN], BF16)
    din("Bmat", [P, JB, EL], BF16)
    din("wemb", [BD, H], BF16)
    din("bemb", [P, HK], F32)
    din("wh", [P, HK, H], BF16)
    din("bh", [P, HK], F32)
    din("inw", [P, L, HK, 3 * H], BF16)
    din("inb", [P, L, M6], F32)
    din("outw", [P, L, HK, H], BF16)
    din("upw", [P, L, HK, H], BF16)
    din("upb2", [L, H], F32)
    din("ln1g", [L, H], F32)
    din("ln1b", [L, H], F32)
    din("ln2g", [L, H], F32)
    din("ln2b", [L, H], F32)
    hout = nc.dram_tensor("hout", [EL, H], F32, kind="ExternalOutput")

    rg = [list(range(NC))]

    with tile.TileContext(nc) as tc:
        with (
            tc.tile_pool(name="const", bufs=1) as const,
            tc.tile_pool(name="sb", bufs=2) as sb,
            tc.tile_pool(name="kv", bufs=1) as kv,
            tc.tile_pool(name="ptp", bufs=4) as ptp,
            tc.tile_pool(name="pmm2", bufs=2, space="PSUM") as pmm2,
            tc.tile_pool(name="pacc", bufs=2, space="PSUM") as pacc,
            tc.tile_pool(name="paux", bufs=2, space="PSUM") as paux,
            tc.tile_pool(name="dram", bufs=1, space="DRAM") as dram,
        ):
            # ---- load constants ----
            bondT_sb = const.tile([BD, EL], BF16)
            nc.sync.dma_start(bondT_sb[:], di["bondT"][:])
            A_sb = const.tile([P, EC, NN], BF16)
            nc.sync.dma_start(A_sb[:], di["Amat"][:])
            B_sb = const.tile([P, JB, EL], BF16)
            nc.sync.dma_start(B_sb[:], di["Bmat"][:])
            wemb_sb = const.tile([BD, H], BF16)
            nc.sync.dma_start(wemb_sb[:], di["wemb"][:])
            bemb_sb = const.tile([P, HK], F32)
            nc.sync.dma_start(bemb_sb[:], di["bemb"][:])
            wh_sb = const.tile([P, HK, H], BF16)
            nc.sync.dma_start(wh_sb[:], di["wh"][:])
            bh_sb = const.tile([P, HK], F32)
            nc.sync.dma_start(bh_sb[:], di["bh"][:])
            inw_sb = const.tile([P, L, HK, 3 * H], BF16)
            nc.sync.dma_start(inw_sb[:], di["inw"][:])
            inb_sb = const.tile([P, L, M6], F32)
            nc.sync.dma_start(inb_sb[:], di["inb"][:])
            outw_sb = const.tile([P, L, HK, H], BF16)
            nc.sync.dma_start(outw_sb[:], di["outw"][:])
            upw_sb = const.tile([P, L, HK, H], BF16)
            nc.sync.dma_start(upw_sb[:], di["upw"][:])

            def bcast_load(name):
                t = const.tile([P, L, H], F32, name=f"{name}_bc")
                src = di[name][:]
                bap = bass.AP(
                    tensor=src.tensor,
                    offset=src.offset,
                    ap=[[0, P]] + [list(x) for x in src.ap],
                )
                nc.sync.dma_start(t[:], bap)
                return t

            upb2_bc = bcast_load("upb2")
            ln1g_bc = bcast_load("ln1g")
            ln1b_bc = bcast_load("ln1b")
            ln2g_bc = bcast_load("ln2g")
            ln2b_bc = bcast_load("ln2b")

            ident_bf = const.tile([P, P], BF16)
            make_identity(nc, ident_bf[:])
            ones_f = const.tile([1, D], F32)
            nc.vector.memset(ones_f[:], 1.0)
            eps_sb = const.tile([P, 1], F32)
            nc.vector.memset(eps_sb[:], 1e-5)

            def transpose_128(dst_ap, src_ap):
                pst = paux.tile([P, P], BF16, tag="aux", name="pst")
                nc.tensor.transpose(pst[:], src_ap, ident_bf[:])
                nc.vector.tensor_copy(dst_ap, pst[:])

            # ---- embedding: h = gelu(bond @ W_emb + b_emb) @ W_h + b_h ----
            g1 = sb.tile([P, HK, EL], BF16, name="g1")
            for m in range(HK):
                ps = paux.tile([P, EL], F32, tag="aux", name="ps_e")
                nc.tensor.matmul(
                    ps[:], wemb_sb[:, m * P:(m + 1) * P], bondT_sb[:],
                    start=True, stop=True,
                )
                nc.scalar.activation(
                    g1[:, m, :], ps[:], AF.Gelu, bias=bemb_sb[:, m:m + 1]
                )
            hT = sb.tile([P, HK, EL], BF16, name="hT")
            for m in range(HK):
                ps = paux.tile([P, EL], F32, tag="aux", name="ps_h")
                for k in range(HK):
                    nc.tensor.matmul(
                        ps[:], wh_sb[:, k, m * P:(m + 1) * P], g1[:, k, :],
                        start=(k == 0), stop=(k == HK - 1),
                    )
                nc.vector.tensor_scalar_add(hT[:, m, :], ps[:], bh_sb[:, m:m + 1])
            h_nat = sb.tile([P, EC, H], BF16, name="h_nat")
            for m in range(HK):
                for c in range(EC):
                    transpose_128(
                        h_nat[:, c, m * P:(m + 1) * P],
                        hT[:, m, c * P:(c + 1) * P],
                    )

            # ---- layers ----
            for t in range(L):
                # A. partial segment-sum over local edges, AllReduce (bf16)
                ar_in = dram.tile([NN, H], BF16, name=f"ar_in{t}")
                ar_out = dram.tile([NN, H], BF16, addr_space="Shared", name=f"ar_out{t}")
                for i in range(NT):
                    ps = paux.tile([P, EL], F32, tag="aux", name="ps_s")
                    for c in range(EC):
                        nc.tensor.matmul(
                            ps[:, :H], A_sb[:, c, i * P:(i + 1) * P], h_nat[:, c, :],
                            start=(c == 0), stop=(c == EC - 1),
                        )
                    s16 = sb.tile([P, H], BF16, tag="s16", name="s16")
                    nc.vector.tensor_copy(s16[:], ps[:, :H])
                    nc.sync.dma_start(ar_in[i * P:(i + 1) * P, :], s16[:])
                nc.gpsimd.collective_compute(
                    "AllReduce", ALU.add, replica_groups=rg,
                    ins=[ar_in[:]], outs=[ar_out[:]],
                )
                s_bf = sb.tile([P, NT, H], BF16, name="s_bf")
                nc.sync.dma_start(
                    s_bf[:],
                    ar_out[:].rearrange("(i p) h -> p i h", p=P),
                )

                # B. r = S[tgt] - deg[tgt]*h  (fused matmul), keep f32
                r_nat = sb.tile([P, EC, H], F32, name="r_nat")
                mv4 = sb.tile([P, EC, 2], F32, name="mv4")
                for m in range(EC):
                    ps = paux.tile([P, EL], F32, tag="aux", name="ps_r")
                    for j in range(JB):
                        rhs = s_bf[:, j, :] if j < NT else h_nat[:, j - NT, :]
                        nc.tensor.matmul(
                            ps[:, :H], B_sb[:, j, m * P:(m + 1) * P], rhs,
                            start=(j == 0), stop=(j == JB - 1),
                        )
                    nc.vector.tensor_copy(r_nat[:, m, :], ps[:, :H])
                    stats = sb.tile([P, 6], F32, tag="stats", name="stats")
                    nc.vector.bn_stats(stats[:], ps[:, :H])
                    nc.vector.bn_aggr(mv4[:, m, :], stats[:])
                # C. LN1 -> xn (bf16): batched rstd then apply
                rstd4 = sb.tile([P, EC], F32, name="rstd4")
                nc.scalar.activation(rstd4[:], mv4[:, :, 1], AF.Sqrt, bias=eps_sb[:])
                nc.vector.reciprocal(rstd4[:], rstd4[:])
                xn_bf = sb.tile([P, EC, H], BF16, name="xn_bf")
                for m in range(EC):
                    xc = sb.tile([P, H], F32, tag="xn32", name="xn32")
                    nc.vector.tensor_scalar(
                        xc[:], r_nat[:, m, :], mv4[:, m, 0:1], rstd4[:, m:m + 1],
                        op0=ALU.subtract, op1=ALU.mult,
                    )
                    nc.vector.tensor_mul(xc[:], xc[:], ln1g_bc[:, t, :])
                    nc.vector.tensor_add(xn_bf[:, m, :], xc[:], ln1b_bc[:, t, :])

                # D. xn^T
                xnT = sb.tile([P, HK, EL], BF16, name="xnT")
                for c in range(EC):
                    for hf in range(HK):
                        transpose_128(
                            xnT[:, hf, c * P:(c + 1) * P],
                            xn_bf[:, c, hf * P:(hf + 1) * P],
                        )

                # E. in-proj: qkv^T = in_w^T @ xn^T + in_b
                QT = sb.tile([P, HK, EL], BF16, name="QT")
                KTl = sb.tile([P, HK, EL], BF16, name="KTl")
                VTl = sb.tile([P, HK, EL], BF16, name="VTl")
                dests = [(QT, 0), (QT, 1), (KTl, 0), (KTl, 1), (VTl, 0), (VTl, 1)]
                for m in range(M6):
                    ps = paux.tile([P, EL], F32, tag="aux", name="ps_q")
                    for k in range(HK):
                        nc.tensor.matmul(
                            ps[:], inw_sb[:, t, k, m * P:(m + 1) * P], xnT[:, k, :],
                            start=(k == 0), stop=(k == HK - 1),
                        )
                    dt_, idx = dests[m]
                    nc.vector.tensor_scalar_add(
                        dt_[:, idx, :], ps[:], inb_sb[:, t, m:m + 1]
                    )

                # F. v natural + ones column (V_aug), local shard
                vnat = sb.tile([P, EC, NH, 33], BF16, name="vnat")
                for hf in range(HK):
                    for c in range(EC):
                        pst = paux.tile([P, P], BF16, tag="aux", name="pst_v")
                        nc.tensor.transpose(
                            pst[:], VTl[:, hf, c * P:(c + 1) * P], ident_bf[:]
                        )
                        nc.vector.tensor_copy(
                            vnat[:, c, hf * 4:(hf + 1) * 4, 0:32],
                            pst[:].rearrange("p (a b) -> p a b", a=4),
                        )
                nc.vector.memset(vnat[:, :, :, 32:33], 1.0)

                # G. AllGather K^T and V_aug
                ag_in = dram.tile([P, AGW], BF16, name=f"ag_in{t}")
                ag_out = dram.tile(
                    [P * NC, AGW], BF16, addr_space="Shared", name=f"ag_out{t}"
                )
                nc.sync.dma_start(
                    ag_in[:, 0:1024].rearrange("p (a b) -> p a b", a=HK), KTl[:]
                )
                nc.sync.dma_start(
                    ag_in[:, 1024:AGW].rearrange(
                        "p (a b c) -> p a b c", a=EC, b=NH
                    ),
                    vnat[:],
                )
                nc.gpsimd.collective_compute(
                    "AllGather", ALU.bypass, replica_groups=rg,
                    ins=[ag_in[:]], outs=[ag_out[:]],
                )
                # shard 0 = own (local tiles, no load); shards 1..7 loaded from
                # the gathered buffer with a partition-id rotated row offset so
                # every core skips its own block uniformly
                me = nc.sync.partition_id()
                KT_s = [KTl]
                V_s = [vnat]
                for j in range(1, NC):
                    row = ((me + j) % NC) * P
                    kts = kv.tile([P, HK, EL], BF16, name=f"kt{j}", tag=f"kt{j}")
                    nc.sync.dma_start(
                        kts[:],
                        ag_out[bass.ds(row, P), 0:1024].rearrange(
                            "p (a b) -> p a b", a=HK
                        ),
                    )
                    vs = kv.tile([P, EC, NH, 33], BF16, name=f"v{j}", tag=f"v{j}")
                    nc.sync.dma_start(
                        vs[:],
                        ag_out[bass.ds(row, P), 1024:AGW].rearrange(
                            "p (a b c) -> p a b c", a=EC, b=NH
                        ),
                    )
                    KT_s.append(kts)
                    V_s.append(vs)

                # zero-padded K=64 Q tiles: head h occupies its 32 rows inside
                # a 64-row block, the partner head's rows are zero, so K=64
                # matmuls at row blocks 0/64 overlap AND keep the PE activity
                # monitor warm (K=32 streams never unthrottle the clock)
                QTz = sb.tile([P, HK, 2, EL], BF16, name="QTz")
                nc.vector.memset(QTz[:], 0.0)
                for h in range(NH):
                    hp = (h % 4) * D
                    hf = h // 4
                    ver = (h % 4) % 2
                    nc.vector.tensor_copy(
                        QTz[hp:hp + D, hf, ver, :], QT[hp:hp + D, hf, :]
                    )

                # H. attention: pairs of heads on disjoint PE quadrants so the
                #    K=32 QK matmuls overlap in the array; scores for the pair
                #    share one 2-bank PSUM tile -> single batched exp; PV with
                #    ones row gives the softmax denominator.
                oT = sb.tile([P, HK, EL], BF16, name="oT")
                for hA, hB in ((0, 2), (1, 3), (4, 6), (5, 7)):
                    pair = (hA, hB)
                    # two accumulators in separate banks at disjoint PE array
                    # columns (0 / 64) so the pair's PV matmuls overlap
                    accs = [
                        pacc.tile([P, EL], F32, tag="acc", name="ps_o")
                        for _ in range(2)
                    ]
                    for kt in range(KT):
                        s, c = divmod(kt, EC)
                        ps2 = pmm2.tile([P, 2, EL], F32, tag="mm", name="ps2")
                        for j, h in enumerate(pair):
                            hf = h // 4
                            hb = ((h % 4) // 2) * 64
                            ver = (h % 4) % 2
                            nc.tensor.matmul(
                                ps2[:, j, :],
                                KT_s[s][hb:hb + 64, hf, c * P:(c + 1) * P],
                                QTz[hb:hb + 64, hf, ver, :],
                                start=True, stop=True,
                                tile_position=(hb, 0),
                            )
                        pt2 = ptp.tile([P, 2, EL], BF16, tag="pt", name="pt")
                        nc.scalar.activation(pt2[:], ps2[:], AF.Exp)
                        for j, h in enumerate(pair):
                            cb = j * 64
                            nc.tensor.matmul(
                                accs[j][cb:cb + 33, :],
                                V_s[s][:, c, h, 0:33], pt2[:, j, :],
                                start=(kt == 0), stop=(kt == KT - 1),
                                tile_position=(0, cb),
                            )
                    for j, h in enumerate(pair):
                        hp = (h % 4) * D
                        hf = h // 4
                        cb = j * 64
                        dnr = sb.tile([1, EL], F32, tag="dnr", name="dnr")
                        nc.vector.tensor_copy(dnr[:], accs[j][cb + 32:cb + 33, :])
                        den = sb.tile([1, EL], F32, tag="den", name="den")
                        nc.vector.reciprocal_approx_fast(den[:], dnr[:])
                        rec32 = sb.tile([D, EL], F32, tag="rec32", name="rec32")
                        nc.gpsimd.partition_broadcast(rec32[:], den[:])
                        nc.vector.tensor_mul(
                            oT[hp:hp + D, hf, :], accs[j][cb:cb + 32, :], rec32[:]
                        )

                # I. out-proj + residual: t_ij = attn + 2r (out_b folded into up_b)
                t_bf = sb.tile([P, EC, H], BF16, name="t_bf")
                for m in range(EC):
                    ps = paux.tile([P, EL], F32, tag="aux", name="ps_a")
                    for k in range(HK):
                        nc.tensor.matmul(
                            ps[:, :H], oT[:, k, m * P:(m + 1) * P], outw_sb[:, t, k, :],
                            start=(k == 0), stop=(k == HK - 1),
                        )
                    nc.vector.scalar_tensor_tensor(
                        t_bf[:, m, :], r_nat[:, m, :], 2.0, ps[:, :H],
                        op0=ALU.mult, op1=ALU.add,
                    )

                # J. t^T
                tT = sb.tile([P, HK, EL], BF16, name="tT")
                for c in range(EC):
                    for hf in range(HK):
                        transpose_128(
                            tT[:, hf, c * P:(c + 1) * P],
                            t_bf[:, c, hf * P:(hf + 1) * P],
                        )

                # K. up-proj + LN2 + gelu -> next h (or output)
                last = t == L - 1
                if not last:
                    h_nat_new = sb.tile([P, EC, H], BF16, name="h_nat")
                u4 = sb.tile([P, EC, H], F32, name="u4")
                mv4b = sb.tile([P, EC, 2], F32, name="mv4b")
                for m in range(EC):
                    ps = paux.tile([P, EL], F32, tag="aux", name="ps_u")
                    for k in range(HK):
                        nc.tensor.matmul(
                            ps[:, :H], tT[:, k, m * P:(m + 1) * P], upw_sb[:, t, k, :],
                            start=(k == 0), stop=(k == HK - 1),
                        )
                    nc.vector.tensor_add(u4[:, m, :], ps[:, :H], upb2_bc[:, t, :])
                    stats = sb.tile([P, 6], F32, tag="stats", name="stats")
                    nc.vector.bn_stats(stats[:], u4[:, m, :])
                    nc.vector.bn_aggr(mv4b[:, m, :], stats[:])
                rstd4b = sb.tile([P, EC], F32, name="rstd4b")
                nc.scalar.activation(rstd4b[:], mv4b[:, :, 1], AF.Sqrt, bias=eps_sb[:])
                nc.vector.reciprocal(rstd4b[:], rstd4b[:])
                for m in range(EC):
                    xc = sb.tile([P, H], F32, tag="xln", name="xln")
                    nc.vector.tensor_scalar(
                        xc[:], u4[:, m, :], mv4b[:, m, 0:1], rstd4b[:, m:m + 1],
                        op0=ALU.subtract, op1=ALU.mult,
                    )
                    nc.vector.tensor_mul(xc[:], xc[:], ln2g_bc[:, t, :])
                    uln = sb.tile([P, H], F32, tag="uln", name="uln")
                    nc.vector.tensor_add(uln[:], xc[:], ln2b_bc[:, t, :])
                    if last:
                        hf32 = sb.tile([P, H], F32, tag="hf32", name="hf32")
                        nc.scalar.activation(hf32[:], uln[:], AF.Gelu)
                        nc.sync.dma_start(hout[m * P:(m + 1) * P, :], hf32[:])
                    else:
                        nc.scalar.activation(h_nat_new[:, m, :], uln[:], AF.Gelu)
                if not last:
                    h_nat = h_nat_new

    nc.compile()
    return nc


_NC_CACHE = None


def _get_nc():
    global _NC_CACHE
    if _NC_CACHE is None:
        _NC_CACHE = _build()
    return _NC_CACHE


def _prepare_in_maps(inputs):
    ei = np.asarray(inputs["edge_index"])
    bond = np.asarray(inputs["bond_features"], dtype=np.float32)
    W_emb = np.asarray(inputs["W_emb"], dtype=np.float32)
    b_emb = np.asarray(inputs["b_emb"], dtype=np.float32)
    W_h = np.asarray(inputs["W_h"], dtype=np.float32)
    b_h = np.asarray(inputs["b_h"], dtype=np.float32)
    ln1_g = np.asarray(inputs["ln1_g"], dtype=np.float32)
    ln1_b = np.asarray(inputs["ln1_b"], dtype=np.float32)
    in_w = np.asarray(inputs["in_w"], dtype=np.float32)
    in_b = np.asarray(inputs["in_b"], dtype=np.float32)
    out_w = np.asarray(inputs["out_w"], dtype=np.float32)
    out_b = np.asarray(inputs["out_b"], dtype=np.float32)
    up_w = np.asarray(inputs["up_w"], dtype=np.float32)
    up_b = np.asarray(inputs["up_b"], dtype=np.float32)
    ln2_g = np.asarray(inputs["ln2_g"], dtype=np.float32)
    ln2_b = np.asarray(inputs["ln2_b"], dtype=np.float32)

    tgt = ei[1].astype(np.int64)
    deg = np.zeros(NN, np.float32)
    np.add.at(deg, tgt, 1.0)
    deg_tgt = deg[tgt]  # [E]

    # scale q columns by 1/sqrt(d)
    sc = 1.0 / np.sqrt(np.float32(D))
    in_w_s = in_w.copy()
    in_w_s[:, :, :H] *= sc
    in_b_s = in_b.copy()
    in_b_s[:, :H] *= sc

    shared = {
        "wemb": W_emb.astype(BFNP),
        "bemb": b_emb.reshape(HK, P).T.copy(),
        "wh": W_h.reshape(HK, P, H).transpose(1, 0, 2).astype(BFNP),
        "bh": b_h.reshape(HK, P).T.copy(),
        "inw": in_w_s.reshape(L, HK, P, 3 * H).transpose(2, 0, 1, 3).astype(BFNP),
        "inb": in_b_s.reshape(L, M6, P).transpose(2, 0, 1).copy(),
        "outw": out_w.reshape(L, HK, P, H).transpose(2, 0, 1, 3).astype(BFNP),
        "upw": up_w.reshape(L, HK, P, H).transpose(2, 0, 1, 3).astype(BFNP),
        "upb2": (up_b + np.einsum("lh,lho->lo", out_b, up_w)).astype(np.float32),
        "ln1g": ln1_g, "ln1b": ln1_b, "ln2g": ln2_g, "ln2b": ln2_b,
    }
    shared = {k: np.ascontiguousarray(v) for k, v in shared.items()}

    in_maps = []
    for c in range(NC):
        sl = slice(c * EL, (c + 1) * EL)
        tl = tgt[sl]
        dl = deg_tgt[sl]
        A = np.zeros((EL, NN), np.float32)
        A[np.arange(EL), tl] = 1.0
        B = np.zeros(((NT + EC) * P, EL), np.float32)
        B[tl, np.arange(EL)] = 1.0
        B[NN + np.arange(EL), np.arange(EL)] = -dl
        m = {
            "bondT": np.ascontiguousarray(bond[sl].T.astype(BFNP)),
            "Amat": np.ascontiguousarray(
                A.reshape(EC, P, NN).transpose(1, 0, 2).astype(BFNP)
            ),
            "Bmat": np.ascontiguousarray(
                B.reshape(JB, P, EL).transpose(1, 0, 2).astype(BFNP)
            ),
        }
        m.update(shared)
        in_maps.append(m)
    return in_maps


def kernel(**inputs):
    nc = _get_nc()
    in_maps = _prepare_in_maps(inputs)
    res = run_bass_kernel_spmd(nc, in_maps, core_ids=list(range(NC)))
    out = np.concatenate(
        [np.asarray(res.results[c]["hout"]) for c in range(NC)], axis=0
    )
    return out.astype(np.float32)
